# revision 1
# baseline (speedup 1.0000x reference)
"""BSplineKAN layer kernel for 8 Trainium2 NeuronCores.

Math
----
The reference computes, per element x = clip(x, -1, 1):
    y[n,o] = sum_{i,b} basis_b(x[n,i]) * coeff[o,i,b]  +  silu(x) @ w_base.T + bias
where basis is the 7-function clamped cubic B-spline basis on knots
{-1(x4), -0.5, 0, 0.5, 1(x4)}.  A quirk of the reference recurrence: at
x == 1.0 exactly (all clamped x >= 1 inputs) the basis row is all ZERO.

On [-1, 1) the basis functions are C^2 piecewise cubics with breakpoints at
+-0.5; we represent them exactly in a two-window local feature basis: for
each half H in {L: [-1,0), R: [0,1)} with center c_H = -+0.5, u = x - c_H,
window mask m_H, and knot-side mask g_H:
    feats_H = [m_H, m_H*u, m_H*u^2, m_H*u^3, g_H*u^3]
All ten features vanish at x == 1 (masks exclude it), reproducing the
reference's edge behavior exactly.  basis_b = M[f,b] @ feats (M integer/48,
exact).  M is folded into coeff on the host and silu/w_base appended as an
11th feature, giving one fused fp16 matmul
    y[n,o] = sum_{i,f} F_f(x[n,i]) * W[f,i,o] + bias
with K = 11*1024 = 11264.  Features are local (|u| <= 0.5), so the
contraction has no large-term cancellation; fp16 operands with fp32 PSUM
accumulation give ~5e-4 scale-relative absmax error (validated vs fp64).
Masks are exact in fp16 and the u-chain rounds at most 3 times, so the
all-fp16 feature pipeline adds no measurable error.

Distribution: 4-way batch x 2-way d_out mesh over 8 cores.  Per core:
x arrives host-transposed as (1024, 2048) fp32 (transposing on host is part
of sharding and keeps TensorE free of transposes), W-shard (11264, 512)
fp16 stays resident in SBUF, output (2048, 512) fp32.  Features are
computed on DVE (fp16 chain, 2x/4x modes) + ACT (affine/square/silu), and
TensorE runs back-to-back 88-tile K-accumulations into PSUM.
"""

import numpy as np

# ---- problem constants (hardcoded per contract) ----
N_FULL, D_IN, D_OUT = 8192, 1024, 1024
MESH_N, MESH_O = 4, 2                 # 4-way batch x 2-way d_out
N_SHARD = N_FULL // MESH_N            # 2048
O_SHARD = D_OUT // MESH_O             # 512
P = 128
NF = 11                               # 10 spline features + silu
IB = D_IN // P                        # 8 i-blocks
KT = IB * NF                          # 88 K-tiles
NCHUNK = 256                          # batch cols per pipeline chunk
NSUB = NCHUNK // P                    # 2
CHUNKS = N_SHARD // NCHUNK            # 8

# basis_b = sum_f feats_f * M[f, b];  feats order:
# [mL, mL*uL, mL*uL^2, mL*uL^3, gL*uL^3, mR, mR*uR, mR*uR^2, mR*uR^3, gR*uR^3]
_M48 = np.array([
    [0,    12,   28,   8,    0,    0,    0],
    [0,   -72,   24,   48,   0,    0,    0],
    [0,    144, -240,  96,   0,    0,    0],
    [-384, 672, -352,  64,   0,    0,    0],
    [384, -768,  576, -256,  64,   0,    0],
    [0,    0,    0,    8,    28,   12,   0],
    [0,    0,    0,   -48,  -24,   72,   0],
    [0,    0,    0,    96,  -240,  144,  0],
    [0,    0,   -64,   192, -224,  96,   0],
    [0,    0,    64,  -256,  576, -768,  384],
], dtype=np.float64)

_PROGRAM = None  # compiled Bass program, built once


def _build_program():
    import concourse.mybir as mybir
    import concourse.tile as tile
    from concourse import bacc

    f32 = mybir.dt.float32
    f16 = mybir.dt.float16
    Op = mybir.AluOpType
    Act = mybir.ActivationFunctionType

    nc = bacc.Bacc("TRN2", target_bir_lowering=False, debug=False)
    xt_d = nc.dram_tensor("xt", [D_IN, N_SHARD], f32, kind="ExternalInput").ap()
    w_d = nc.dram_tensor("wt", [KT * P, O_SHARD], f16, kind="ExternalInput").ap()
    b_d = nc.dram_tensor("biasb", [P, O_SHARD], f32, kind="ExternalInput").ap()
    y_d = nc.dram_tensor("y", [N_SHARD, O_SHARD], f32, kind="ExternalOutput").ap()

    with tile.TileContext(nc) as tc:
        with (
            tc.tile_pool(name="const", bufs=1) as const_pool,
            tc.tile_pool(name="wt", bufs=1) as wt_pool,
            tc.tile_pool(name="feat", bufs=2) as f_pool,
            tc.tile_pool(name="xc", bufs=2) as xc_pool,
            tc.tile_pool(name="tmp", bufs=2) as tmp_pool,
            tc.tile_pool(name="out", bufs=1) as out_pool,
            tc.tile_pool(name="pso", bufs=4, space="PSUM") as psum_out,
        ):
            bias_s = const_pool.tile([P, O_SHARD], f32)
            # tiny dummy activations up front so both ACT table sets load
            # concurrently with the initial DMAs instead of on the first
            # feature's critical path
            warm = const_pool.tile([P, 1], f32, name="warm")
            nc.gpsimd.memset(warm[:], 0.0)
            nc.scalar.activation(warm[:], warm[:], Act.Copy, bias=0.0)
            nc.scalar.activation(warm[:], warm[:], Act.Square)
            nc.scalar.activation(warm[:], warm[:], Act.Silu)
            b05 = const_pool.tile([P, 1], f32, name="b05")
            nc.gpsimd.memset(b05[:], 0.5)
            bm05 = const_pool.tile([P, 1], f32, name="bm05")
            nc.gpsimd.memset(bm05[:], -0.5)

            # warm-up: tiny matmuls on a zeroed tile fill the initial DMA
            # wait so the PE clock (HAM) is at full rate when the first real
            # matmul issues
            wz = const_pool.tile([P, P], f16, name="wz")
            nc.gpsimd.memset(wz[:], 0.0)
            pw = psum_out.tile([P, 64], f32, tag="pwarm", name="pwarm")
            for i in range(185):
                nc.tensor.matmul(pw[:], wz[:], wz[:, :64],
                                 start=(i == 0), stop=(i == 184))

            # startup DMA order minimizes time-to-first-real-matmul on the
            # serial queue: first half of chunk-0 x (enough for i-blocks 0-3),
            # then the first two weight slabs, then the rest of x, then the
            # remaining slabs.  One DMA per ib-slab of 11 weight tiles: HWDGE
            # charges per DMA instruction, so batching keeps the queue off
            # the critical path during the initial weight stream.
            xt_r = xt_d.rearrange("(ib p) n -> p ib n", p=P)
            xc0 = xc_pool.tile([P, IB, NCHUNK], f32, tag="xc", name="xc0")
            nc.sync.dma_start(xc0[:, :2], xt_r[:, :2, 0:NCHUNK])

            wt = {}
            def load_wt(ib):
                t = wt_pool.tile([P, NF, O_SHARD], f16, tag=f"wt_{ib}", name=f"wt_{ib}")
                r0 = ib * NF * P
                nc.sync.dma_start(
                    t[:], w_d[r0:r0 + NF * P, :].rearrange("(f p) o -> p f o", p=P))
                wt[ib] = t
            load_wt(0)
            load_wt(1)
            nc.sync.dma_start(xc0[:, 2:], xt_r[:, 2:, 0:NCHUNK])
            for ib in range(2, IB - 1):
                load_wt(ib)
            # chunk-1's x jumps ahead of the last weight slab (slab 7 is not
            # consumed until ~41us) so chunk-1 features start early
            xc1 = xc_pool.tile([P, IB, NCHUNK], f32, tag="xc", name="xc1")
            nc.sync.dma_start(xc1[:], xt_r[:, :, NCHUNK:2 * NCHUNK])
            load_wt(IB - 1)

            # bias is first needed at the first eviction (~30us in); loading
            # it after the weight slabs keeps it off the startup critical path
            nc.sync.dma_start(bias_s[:], b_d[:])

            for chunk in range(CHUNKS):
                c0 = chunk * NCHUNK
                # -- load x^T slices, clamp, build fp16 features --
                F = {}
                if chunk == 0:
                    # clamp in pieces matching the split chunk-0 DMAs so early
                    # i-blocks don't wait on later halves' arrival
                    xch = xc0
                    nc.vector.tensor_scalar(xch[:, :2], xch[:, :2],
                                            -1.0, 1.0, Op.max, Op.min)
                    nc.vector.tensor_scalar(xch[:, 2:], xch[:, 2:],
                                            -1.0, 1.0, Op.max, Op.min)
                else:
                    if chunk == 1:
                        xch = xc1
                    else:
                        xch = xc_pool.tile([P, IB, NCHUNK], f32, tag="xc", name="xc")
                        nc.sync.dma_start(xch[:], xt_r[:, :, c0:c0 + NCHUNK])
                    nc.vector.tensor_scalar(xch[:], xch[:], -1.0, 1.0, Op.max, Op.min)
                for ib in range(IB):
                    xcb = xch[:, ib]

                    def tmp(tag, w=2, bufs=2):
                        return tmp_pool.tile([P, w * NCHUNK], f16, tag=tag, name=tag, bufs=bufs)

                    def fpair(f):
                        # (128, 2*NCHUNK) tile holding K-tiles (ib, f) on the
                        # left half and (ib, f+5) on the right half
                        t = f_pool.tile([P, 2 * NCHUNK], f16, tag=f"F_{ib}_{f}",
                                        name=f"F_{ib}_{f}")
                        F[ib, f] = t
                        return t

                    N = NCHUNK
                    # cumulative masks on GpSimd (1-input ops run near line-rate there)
                    cB = tmp("cB", 1); nc.gpsimd.tensor_scalar(cB[:], xcb[:], 0.0, None, Op.is_ge)
                    cD = tmp("cD", 1, 1); nc.gpsimd.tensor_scalar(cD[:], xcb[:], 1.0, None, Op.is_ge)
                    # window masks (exact 0/1 in fp16): Fm = [mL | mR]
                    Fm = fpair(0)
                    nc.gpsimd.tensor_scalar(Fm[:, :N], xcb[:], 0.0, None, Op.is_lt)
                    nc.vector.tensor_tensor(Fm[:, N:], cB[:], cD[:], Op.subtract)
                    # u-chain: ACT writes both halves from the same xcb
                    U = tmp("U")
                    nc.scalar.activation(U[:, :N], xcb[:], Act.Copy, bias=0.5)
                    nc.scalar.activation(U[:, N:], xcb[:], Act.Copy, bias=-0.5)
                    U2 = tmp("U2")
                    nc.scalar.activation(U2[:, :N], xcb[:], Act.Square, bias=b05[:])
                    nc.scalar.activation(U2[:, N:], xcb[:], Act.Square, bias=bm05[:])
                    U3 = tmp("U3")
                    nc.vector.tensor_tensor(U3[:], U2[:], U[:], Op.mult)
                    # windowed monomials: one 512-wide fp16 op per L/R pair
                    nc.vector.tensor_tensor(fpair(1)[:], Fm[:], U[:], Op.mult)
                    nc.vector.tensor_tensor(fpair(2)[:], Fm[:], U2[:], Op.mult)
                    nc.vector.tensor_tensor(fpair(3)[:], Fm[:], U3[:], Op.mult)
                    # knot-side features: g_H * u^3 == m_H * relu(u^3), fused
                    nc.vector.scalar_tensor_tensor(fpair(4)[:], U3[:], 0.0, Fm[:],
                                                   Op.max, Op.mult)
                    fs = f_pool.tile([P, NCHUNK], f16, tag=f"F_{ib}_s", name=f"F_{ib}_s")
                    F[ib, 10] = fs
                    nc.scalar.activation(fs[:], xcb[:], Act.Silu)

                # -- matmuls. Chunk 0 runs k-major over both 128-batch
                # subtiles so each weight slab feeds two matmuls the moment its
                # DMA lands (PE stays ahead of the initial weight stream);
                # later chunks run the subtiles serially so group-0's eviction
                # overlaps group-1's matmuls --
                def lhs(ib, f, ns):
                    if f == 10:
                        return F[ib, 10][:, ns * P:(ns + 1) * P]
                    if f < 5:
                        return F[ib, f][:, ns * P:(ns + 1) * P]
                    return F[ib, f - 5][:, NCHUNK + ns * P:NCHUNK + (ns + 1) * P]

                def evict(ps, ns):
                    o = out_pool.tile([P, O_SHARD], f32, tag="out", name="outt")
                    nc.vector.tensor_tensor(o[:], ps[:], bias_s[:], Op.add)
                    r0 = c0 + ns * P
                    nc.sync.dma_start(y_d[r0:r0 + P, :], o[:])

                if chunk == 0:
                    pss = [psum_out.tile([P, O_SHARD], f32, tag=f"psout{ns}",
                                         name=f"psout{ns}", bufs=2)
                           for ns in range(NSUB)]
                    for k, (ib, f) in enumerate(
                            (ib, f) for ib in range(IB) for f in range(NF)):
                        for ns in range(NSUB):
                            nc.tensor.matmul(
                                pss[ns][:], lhs(ib, f, ns), wt[ib][:, f],
                                start=(k == 0), stop=(k == KT - 1))
                    for ns in range(NSUB):
                        evict(pss[ns], ns)
                else:
                    for ns in range(NSUB):
                        ps = psum_out.tile([P, O_SHARD], f32, tag=f"psout{ns}",
                                           name=f"psout{ns}", bufs=2)
                        for k, (ib, f) in enumerate(
                                (ib, f) for ib in range(IB) for f in range(NF)):
                            nc.tensor.matmul(
                                ps[:], lhs(ib, f, ns), wt[ib][:, f],
                                start=(k == 0), stop=(k == KT - 1))
                        evict(ps, ns)

    nc.compile()
    return nc


def _fold_weights(coeff, w_base):
    """Fold the feature->basis matrix into coeff; returns (K, D_OUT) fp16."""
    M = _M48 / 48.0
    c64 = np.asarray(coeff).astype(np.float64)
    # Wf[f, i, o] = sum_b M[f, b] * coeff[o, i, b]
    Wf = np.einsum('fb,oib->fio', M, c64)
    W11 = np.concatenate([Wf, np.asarray(w_base).astype(np.float64).T[None]], axis=0)  # (11, i, o)
    # pack K as (ib, f, p): row k = ib*(NF*P) + f*P + p  <->  W11[f, ib*P+p, o]
    Wt = W11.reshape(NF, IB, P, D_OUT).transpose(1, 0, 2, 3).reshape(KT * P, D_OUT)
    return Wt.astype(np.float16)


def kernel(x, coeff, w_base, bias):
    global _PROGRAM
    from concourse.bass_utils import run_bass_kernel_spmd

    if _PROGRAM is None:
        _PROGRAM = _build_program()
    nc = _PROGRAM

    x = np.asarray(x, dtype=np.float32)
    Wt = _fold_weights(coeff, w_base)
    bias = np.asarray(bias, dtype=np.float32)

    in_maps = []
    for core in range(8):
        cn, co = divmod(core, MESH_O)
        in_maps.append({
            "xt": np.ascontiguousarray(x[cn * N_SHARD:(cn + 1) * N_SHARD].T),
            "wt": np.ascontiguousarray(Wt[:, co * O_SHARD:(co + 1) * O_SHARD]),
            "biasb": np.ascontiguousarray(np.broadcast_to(
                bias[co * O_SHARD:(co + 1) * O_SHARD], (P, O_SHARD)).astype(np.float32)),
        })

    res = run_bass_kernel_spmd(nc, in_maps, list(range(8)))

    y = np.empty((N_FULL, D_OUT), dtype=np.float32)
    for core in range(8):
        cn, co = divmod(core, MESH_O)
        y[cn * N_SHARD:(cn + 1) * N_SHARD, co * O_SHARD:(co + 1) * O_SHARD] = \
            res.results[core]["y"]
    return y



# revision 2
# speedup vs baseline: 1.3460x; 1.3460x over previous
"""BSplineKAN layer kernel for 8 Trainium2 NeuronCores.

Math
----
The reference computes, per element x = clip(x, -1, 1):
    y[n,o] = sum_{i,b} basis_b(x[n,i]) * coeff[o,i,b]  +  silu(x) @ w_base.T + bias
where basis is the 7-function clamped cubic B-spline basis on knots
{-1(x4), -0.5, 0, 0.5, 1(x4)}.  A quirk of the reference recurrence: at
x == 1.0 exactly (all clamped x >= 1 inputs) the basis row is all ZERO.

On [-1, 1) the 7 basis functions are C^2 piecewise cubics with breakpoints
at {-0.5, 0, +0.5}; the 7-dim space they span has the center-anchored
truncated-power basis
    feats = [m, m*x, m*x^2, m*x^3, m*x_+^3, m*(x-1/2)_+^3, min(x+1/2,0)^3]
with m = (x < 1) the edge mask (all seven vanish at x == 1, reproducing
the reference's edge behavior exactly; min(x+1/2,0)^3 is zero at x=1 on
its own).  basis_b = T[f,b] @ feats with T integer/6, exact.  T is folded
into coeff on the host and silu appended as an 8th feature, giving one
fused fp16 matmul
    y[n,o] = sum_{i,f} F_f(x[n,i]) * W[f,i,o] + bias
with K = 8*1024 = 8192 (vs 11*1024 for the two-half-window local basis:
the global basis trades a ~3x larger fp16 cancellation error - still
~2e-3 scale-relative absmax, 10x under the gate - for 27% fewer FLOPs).
fp16 operands with fp32 PSUM accumulation.

Distribution: 4-way batch x 2-way d_out mesh over 8 cores.  Per core:
x arrives host-transposed as (1024, 2048) fp32 (transposing on host is part
of sharding and keeps TensorE free of transposes), W-shard (8192, 512)
fp16 stays resident in SBUF, output (2048, 512) fp32.  Features are
computed on DVE/ACT/Pool, and TensorE runs back-to-back 64-tile
K-accumulations into PSUM.
"""

import numpy as np

# ---- problem constants (hardcoded per contract) ----
N_FULL, D_IN, D_OUT = 8192, 1024, 1024
MESH_N, MESH_O = 4, 2                 # 4-way batch x 2-way d_out
N_SHARD = N_FULL // MESH_N            # 2048
O_SHARD = D_OUT // MESH_O             # 512
P = 128
NF = 8                                # 7 spline features + silu
IB = D_IN // P                        # 8 i-blocks
KT = IB * NF                          # 64 K-tiles
NCHUNK = 256                          # batch cols per pipeline chunk
NSUB = NCHUNK // P                    # 2
CHUNKS = N_SHARD // NCHUNK            # 8

# basis_b = sum_f feats_f * T6[f, b] / 6; feats order:
# [m, m*x, m*x^2, m*x^3, m*relu(x)^3, m*relu(x-1/2)^3, min(x+1/2,0)^3]
_T6 = np.array([
    [0,    0,    1,    4,    1,    0,   0],
    [0,    0,   -6,    0,    6,    0,   0],
    [0,    0,   12,  -24,   12,    0,   0],
    [0,  -12,   28,  -24,    8,    0,   0],
    [0,   12,  -36,   48,  -36,   12,   0],
    [0,    0,    8,  -32,   72,  -96,  48],
    [-48, 96,  -72,   32,   -8,    0,   0],
], dtype=np.float64)

_PROGRAM = None  # compiled Bass program, built once


def _build_program():
    import concourse.mybir as mybir
    import concourse.tile as tile
    from concourse import bacc

    f32 = mybir.dt.float32
    f16 = mybir.dt.float16
    Op = mybir.AluOpType
    Act = mybir.ActivationFunctionType

    nc = bacc.Bacc("TRN2", target_bir_lowering=False, debug=False)
    xt_d = nc.dram_tensor("xt", [D_IN, N_SHARD], f32, kind="ExternalInput").ap()
    w_d = nc.dram_tensor("wt", [KT * P, O_SHARD], f16, kind="ExternalInput").ap()
    b_d = nc.dram_tensor("biasb", [P, O_SHARD], f32, kind="ExternalInput").ap()
    y_d = nc.dram_tensor("y", [N_SHARD, O_SHARD], f32, kind="ExternalOutput").ap()

    with tile.TileContext(nc) as tc:
        with (
            tc.tile_pool(name="const", bufs=1) as const_pool,
            tc.tile_pool(name="wt", bufs=1) as wt_pool,
            tc.tile_pool(name="feat", bufs=2) as f_pool,
            tc.tile_pool(name="xc", bufs=2) as xc_pool,
            tc.tile_pool(name="tmp", bufs=2) as tmp_pool,
            tc.tile_pool(name="out", bufs=1) as out_pool,
            tc.tile_pool(name="pso", bufs=4, space="PSUM") as psum_out,
        ):
            bias_s = const_pool.tile([P, O_SHARD], f32)
            # tiny dummy activations up front so both ACT table sets load
            # concurrently with the initial DMAs instead of on the first
            # feature's critical path
            warm = const_pool.tile([P, 1], f32, name="warm")
            nc.gpsimd.memset(warm[:], 0.0)
            nc.scalar.activation(warm[:], warm[:], Act.Copy, bias=0.0)
            nc.scalar.activation(warm[:], warm[:], Act.Square)
            nc.scalar.activation(warm[:], warm[:], Act.Silu)
            b05 = const_pool.tile([P, 1], f32, name="b05")
            nc.gpsimd.memset(b05[:], 0.5)
            bm05 = const_pool.tile([P, 1], f32, name="bm05")
            nc.gpsimd.memset(bm05[:], -0.5)

            # warm-up: tiny matmuls on a zeroed tile fill the initial DMA
            # wait so the PE clock (HAM) is at full rate when the first real
            # matmul issues
            wz = const_pool.tile([P, P], f16, name="wz")
            nc.gpsimd.memset(wz[:], 0.0)
            pw = psum_out.tile([P, 64], f32, tag="pwarm", name="pwarm")
            for i in range(185):
                nc.tensor.matmul(pw[:], wz[:], wz[:, :64],
                                 start=(i == 0), stop=(i == 184))

            # startup DMA order minimizes time-to-first-real-matmul on the
            # serial queue: first half of chunk-0 x (enough for i-blocks 0-3),
            # then the first two weight slabs, then the rest of x, then the
            # remaining slabs.  One DMA per ib-slab of 8 weight tiles: HWDGE
            # charges per DMA instruction, so batching keeps the queue off
            # the critical path during the initial weight stream.
            xt_r = xt_d.rearrange("(ib p) n -> p ib n", p=P)
            xc0 = xc_pool.tile([P, IB, NCHUNK], f32, tag="xc", name="xc0")
            nc.sync.dma_start(xc0[:, :2], xt_r[:, :2, 0:NCHUNK])

            wt = {}
            def load_wt(ib):
                t = wt_pool.tile([P, NF, O_SHARD], f16, tag=f"wt_{ib}", name=f"wt_{ib}")
                r0 = ib * NF * P
                nc.sync.dma_start(
                    t[:], w_d[r0:r0 + NF * P, :].rearrange("(f p) o -> p f o", p=P))
                wt[ib] = t
            load_wt(0)
            load_wt(1)
            nc.sync.dma_start(xc0[:, 2:], xt_r[:, 2:, 0:NCHUNK])
            for ib in range(2, IB - 1):
                load_wt(ib)
            # chunk-1's x jumps ahead of the last weight slab (slab 7 is not
            # consumed until late in chunk 0) so chunk-1 features start early
            xc1 = xc_pool.tile([P, IB, NCHUNK], f32, tag="xc", name="xc1")
            nc.sync.dma_start(xc1[:], xt_r[:, :, NCHUNK:2 * NCHUNK])
            load_wt(IB - 1)

            # bias is first needed at the first eviction; loading it after
            # the weight slabs keeps it off the startup critical path
            nc.sync.dma_start(bias_s[:], b_d[:])

            for chunk in range(CHUNKS):
                c0 = chunk * NCHUNK
                # -- load x^T slices, clamp, build fp16 features --
                F = {}
                if chunk == 0:
                    # clamp in pieces matching the split chunk-0 DMAs so early
                    # i-blocks don't wait on later halves' arrival
                    xch = xc0
                    nc.vector.tensor_scalar(xch[:, :2], xch[:, :2],
                                            -1.0, 1.0, Op.max, Op.min)
                    nc.vector.tensor_scalar(xch[:, 2:], xch[:, 2:],
                                            -1.0, 1.0, Op.max, Op.min)
                else:
                    if chunk == 1:
                        xch = xc1
                    else:
                        xch = xc_pool.tile([P, IB, NCHUNK], f32, tag="xc", name="xc")
                        nc.sync.dma_start(xch[:], xt_r[:, :, c0:c0 + NCHUNK])
                    nc.vector.tensor_scalar(xch[:], xch[:], -1.0, 1.0, Op.max, Op.min)
                for ib in range(IB):
                    xcb = xch[:, ib]
                    N = NCHUNK

                    def single(f):
                        t = f_pool.tile([P, NCHUNK], f16, tag=f"F_{ib}_{f}",
                                        name=f"F_{ib}_{f}")
                        F[ib, f] = t
                        return t

                    # A = [x-1/2 | x+1/2], A2 = squares, A3 = cubes (fp16)
                    A = tmp_pool.tile([P, 2 * NCHUNK], f16, tag="A", name="A")
                    nc.scalar.activation(A[:, :N], xcb[:], Act.Copy, bias=-0.5)
                    nc.scalar.activation(A[:, N:], xcb[:], Act.Copy, bias=0.5)
                    A2 = tmp_pool.tile([P, 2 * NCHUNK], f16, tag="A2", name="A2")
                    nc.scalar.activation(A2[:, :N], xcb[:], Act.Square, bias=bm05[:])
                    nc.scalar.activation(A2[:, N:], xcb[:], Act.Square, bias=b05[:])
                    A3 = tmp_pool.tile([P, 2 * NCHUNK], f16, tag="A3", name="A3")
                    nc.vector.tensor_tensor(A3[:], A2[:], A[:], Op.mult)
                    # mask m = (x < 1), exact 0/1 in fp16
                    m = single(0)
                    nc.gpsimd.tensor_scalar(m[:], xcb[:], 1.0, None, Op.is_lt)
                    # f5 = m*relu((x-1/2)^3); f6 = min(x+1/2,0)^3 (self-masked)
                    nc.vector.scalar_tensor_tensor(single(5)[:], A3[:, :N], 0.0,
                                                   m[:], Op.max, Op.mult)
                    nc.gpsimd.tensor_scalar(single(6)[:], A3[:, N:], 0.0, None,
                                            Op.min)
                    # monomial chain: f1 = m*x, f2 = f1^2, f3 = f1*f2,
                    # f4 = relu(f3)
                    f1 = single(1)
                    nc.vector.scalar_tensor_tensor(f1[:], xcb[:], 0.0, m[:],
                                                   Op.add, Op.mult)
                    f2 = single(2)
                    nc.scalar.activation(f2[:], f1[:], Act.Square)
                    f3 = single(3)
                    nc.vector.tensor_tensor(f3[:], f1[:], f2[:], Op.mult)
                    nc.gpsimd.tensor_scalar(single(4)[:], f3[:], 0.0, None, Op.max)
                    # f7 = silu(x)
                    nc.scalar.activation(single(7)[:], xcb[:], Act.Silu)

                # -- matmuls. Chunk 0 runs k-major over both 128-batch
                # subtiles so each weight slab feeds two matmuls the moment its
                # DMA lands (PE stays ahead of the initial weight stream);
                # later chunks run the subtiles serially so group-0's eviction
                # overlaps group-1's matmuls --
                def lhs(ib, f, ns):
                    return F[ib, f][:, ns * P:(ns + 1) * P]

                def evict(ps, ns):
                    o = out_pool.tile([P, O_SHARD], f32, tag="out", name="outt")
                    nc.vector.tensor_tensor(o[:], ps[:], bias_s[:], Op.add)
                    r0 = c0 + ns * P
                    nc.sync.dma_start(y_d[r0:r0 + P, :], o[:])

                if chunk == 0:
                    pss = [psum_out.tile([P, O_SHARD], f32, tag=f"psout{ns}",
                                         name=f"psout{ns}", bufs=2)
                           for ns in range(NSUB)]
                    for k, (ib, f) in enumerate(
                            (ib, f) for ib in range(IB) for f in range(NF)):
                        for ns in range(NSUB):
                            nc.tensor.matmul(
                                pss[ns][:], lhs(ib, f, ns), wt[ib][:, f],
                                start=(k == 0), stop=(k == KT - 1))
                    for ns in range(NSUB):
                        evict(pss[ns], ns)
                else:
                    for ns in range(NSUB):
                        ps = psum_out.tile([P, O_SHARD], f32, tag=f"psout{ns}",
                                           name=f"psout{ns}", bufs=2)
                        for k, (ib, f) in enumerate(
                                (ib, f) for ib in range(IB) for f in range(NF)):
                            nc.tensor.matmul(
                                ps[:], lhs(ib, f, ns), wt[ib][:, f],
                                start=(k == 0), stop=(k == KT - 1))
                        evict(ps, ns)

    nc.compile()
    return nc


def _fold_weights(coeff, w_base):
    """Fold the feature->basis matrix into coeff; returns (K, D_OUT) fp16."""
    T = _T6 / 6.0
    c64 = np.asarray(coeff).astype(np.float64)
    # Wf[f, i, o] = sum_b T[f, b] * coeff[o, i, b]
    Wf = np.einsum('fb,oib->fio', T, c64)
    W8 = np.concatenate([Wf, np.asarray(w_base).astype(np.float64).T[None]], axis=0)
    # pack K as (ib, f, p): row k = ib*(NF*P) + f*P + p  <->  W8[f, ib*P+p, o]
    Wt = W8.reshape(NF, IB, P, D_OUT).transpose(1, 0, 2, 3).reshape(KT * P, D_OUT)
    return Wt.astype(np.float16)


def kernel(x, coeff, w_base, bias):
    global _PROGRAM
    from concourse.bass_utils import run_bass_kernel_spmd

    if _PROGRAM is None:
        _PROGRAM = _build_program()
    nc = _PROGRAM

    x = np.asarray(x, dtype=np.float32)
    Wt = _fold_weights(coeff, w_base)
    bias = np.asarray(bias, dtype=np.float32)

    in_maps = []
    for core in range(8):
        cn, co = divmod(core, MESH_O)
        in_maps.append({
            "xt": np.ascontiguousarray(x[cn * N_SHARD:(cn + 1) * N_SHARD].T),
            "wt": np.ascontiguousarray(Wt[:, co * O_SHARD:(co + 1) * O_SHARD]),
            "biasb": np.ascontiguousarray(np.broadcast_to(
                bias[co * O_SHARD:(co + 1) * O_SHARD], (P, O_SHARD)).astype(np.float32)),
        })

    res = run_bass_kernel_spmd(nc, in_maps, list(range(8)))

    y = np.empty((N_FULL, D_OUT), dtype=np.float32)
    for core in range(8):
        cn, co = divmod(core, MESH_O)
        y[cn * N_SHARD:(cn + 1) * N_SHARD, co * O_SHARD:(co + 1) * O_SHARD] = \
            res.results[core]["y"]
    return y


# revision 3
# speedup vs baseline: 1.3539x; 1.0058x over previous
"""BSplineKAN layer kernel for 8 Trainium2 NeuronCores.

Math
----
The reference computes, per element x = clip(x, -1, 1):
    y[n,o] = sum_{i,b} basis_b(x[n,i]) * coeff[o,i,b]  +  silu(x) @ w_base.T + bias
where basis is the 7-function clamped cubic B-spline basis on knots
{-1(x4), -0.5, 0, 0.5, 1(x4)}.  A quirk of the reference recurrence: at
x == 1.0 exactly (all clamped x >= 1 inputs) the basis row is all ZERO.

On [-1, 1) the 7 basis functions are C^2 piecewise cubics with breakpoints
at {-0.5, 0, +0.5}; the 7-dim space they span has the center-anchored
truncated-power basis
    feats = [m, m*x, m*x^2, m*x^3, m*x_+^3, m*(x-1/2)_+^3, min(x+1/2,0)^3]
with m = (x < 1) the edge mask (all seven vanish at x == 1, reproducing
the reference's edge behavior exactly).  basis_b = T[f,b] @ feats with T
integer/6, exact.  T is folded into coeff on the host and silu appended
as an 8th feature, giving one fused fp16 matmul
    y[n,o] = sum_{i,f} F_f(x[n,i]) * W[f,i,o] + bias
with K = 8*1024 = 8192 (vs 11*1024 for the two-half-window local basis:
the global basis trades a ~3x larger fp16 cancellation error - still
~2e-3 scale-relative absmax, 10x under the gate - for 27% fewer FLOPs).
fp16 operands with fp32 PSUM accumulation.

x ships as fp16, pre-clamped on the host.  Values in (1-2^-12, 1) that
fp16 would round to exactly 1.0 are pinned to 1-2^-11 so the on-chip
mask m = (x < 1) matches the reference's fp32 comparison; this halves
the x DMA and removes the on-chip clamp from every dependency chain.

Distribution: 4-way batch x 2-way d_out mesh over 8 cores.  Per core:
x arrives host-transposed as (1024, 2048) fp16, W-shard (8192, 512)
fp16 stays resident in SBUF, output (2048, 512) fp32.  Features are
computed on DVE/ACT/Pool, and TensorE runs back-to-back 64-tile
K-accumulations into PSUM at the fp16 roofline (~213 ns per
512-column matmul).  Startup DMA order + PE warm-up are tuned so the
first real matmul issues ~4 us in at full clock; evictions are emitted
after the next chunk's feature ops so they never block the boundary.
"""

import numpy as np

# ---- problem constants (hardcoded per contract) ----
N_FULL, D_IN, D_OUT = 8192, 1024, 1024
MESH_N, MESH_O = 4, 2                 # 4-way batch x 2-way d_out
N_SHARD = N_FULL // MESH_N            # 2048
O_SHARD = D_OUT // MESH_O             # 512
P = 128
NF = 8                                # 7 spline features + silu
IB = D_IN // P                        # 8 i-blocks
KT = IB * NF                          # 64 K-tiles
NCHUNK = 256                          # batch cols per pipeline chunk
NSUB = NCHUNK // P                    # 2
CHUNKS = N_SHARD // NCHUNK            # 8
N_WARM = 72                           # PE warm-up matmuls (p-state + DMA cover)

# basis_b = sum_f feats_f * T6[f, b] / 6; feats order:
# [m, m*x, m*x^2, m*x^3, m*relu(x)^3, m*relu(x-1/2)^3, min(x+1/2,0)^3]
_T6 = np.array([
    [0,    0,    1,    4,    1,    0,   0],
    [0,    0,   -6,    0,    6,    0,   0],
    [0,    0,   12,  -24,   12,    0,   0],
    [0,  -12,   28,  -24,    8,    0,   0],
    [0,   12,  -36,   48,  -36,   12,   0],
    [0,    0,    8,  -32,   72,  -96,  48],
    [-48, 96,  -72,   32,   -8,    0,   0],
], dtype=np.float64)

_PROGRAM = None  # compiled Bass program, built once


def _build_program():
    import concourse.mybir as mybir
    import concourse.tile as tile
    from concourse import bacc

    f32 = mybir.dt.float32
    f16 = mybir.dt.float16
    Op = mybir.AluOpType
    Act = mybir.ActivationFunctionType

    nc = bacc.Bacc("TRN2", target_bir_lowering=False, debug=False)
    xt_d = nc.dram_tensor("xt", [D_IN, N_SHARD], f16, kind="ExternalInput").ap()
    w_d = nc.dram_tensor("wt", [KT * P, O_SHARD], f16, kind="ExternalInput").ap()
    b_d = nc.dram_tensor("biasb", [P, O_SHARD], f32, kind="ExternalInput").ap()
    y_d = nc.dram_tensor("y", [N_SHARD, O_SHARD], f32, kind="ExternalOutput").ap()

    with tile.TileContext(nc) as tc:
        with (
            tc.tile_pool(name="const", bufs=1) as const_pool,
            tc.tile_pool(name="wt", bufs=1) as wt_pool,
            tc.tile_pool(name="feat", bufs=2) as f_pool,
            tc.tile_pool(name="xc", bufs=2) as xc_pool,
            tc.tile_pool(name="tmp", bufs=2) as tmp_pool,
            tc.tile_pool(name="out", bufs=3) as out_pool,
            tc.tile_pool(name="pso", bufs=4, space="PSUM") as psum_out,
        ):
            # PE warm-up first: wz memset is the only dependency, so the
            # warm-up matmuls start ~0.4us in and ramp the PE p-state while
            # the startup DMAs stream.
            wz = const_pool.tile([P, P], f16, name="wz")
            nc.gpsimd.memset(wz[:], 0.0)
            pw = psum_out.tile([P, 64], f32, tag="pwarm", name="pwarm")
            for i in range(N_WARM):
                nc.tensor.matmul(pw[:], wz[:], wz[:, :64],
                                 start=(i == 0), stop=(i == N_WARM - 1))

            # tiny dummy activations so both ACT table sets load concurrently
            # with the initial DMAs instead of on the first feature's
            # critical path
            warm = const_pool.tile([P, 1], f32, name="warm")
            nc.gpsimd.memset(warm[:], 0.0)
            nc.scalar.activation(warm[:], warm[:], Act.Copy, bias=0.0)
            nc.scalar.activation(warm[:], warm[:], Act.Square)
            nc.scalar.activation(warm[:], warm[:], Act.Silu)
            b05 = const_pool.tile([P, 1], f32, name="b05")
            nc.gpsimd.memset(b05[:], 0.5)
            bm05 = const_pool.tile([P, 1], f32, name="bm05")
            nc.gpsimd.memset(bm05[:], -0.5)
            bias_s = const_pool.tile([P, O_SHARD], f32)

            # startup DMA order minimizes time-to-first-real-matmul on the
            # serial DMA device: a tiny x slice for i-block 0, then weight
            # slab 0, then the rest of chunk-0 x, then the remaining slabs.
            # Chunk-1 x is split the same way and threaded between the last
            # slabs so chunk-1 features start before the boundary.
            xt_r = xt_d.rearrange("(ib p) n -> p ib n", p=P)
            xc0 = xc_pool.tile([P, IB, NCHUNK], f16, tag="xc", name="xc0")
            nc.sync.dma_start(xc0[:, :1], xt_r[:, :1, 0:NCHUNK])

            wt = {}
            def load_wt(ib):
                t = wt_pool.tile([P, NF, O_SHARD], f16, tag=f"wt_{ib}", name=f"wt_{ib}")
                r0 = ib * NF * P
                nc.sync.dma_start(
                    t[:], w_d[r0:r0 + NF * P, :].rearrange("(f p) o -> p f o", p=P))
                wt[ib] = t
            load_wt(0)
            nc.sync.dma_start(xc0[:, 1:], xt_r[:, 1:, 0:NCHUNK])
            for ib in range(1, IB - 1):
                load_wt(ib)
            xc1 = xc_pool.tile([P, IB, NCHUNK], f16, tag="xc", name="xc1")
            nc.sync.dma_start(xc1[:, :1], xt_r[:, :1, NCHUNK:2 * NCHUNK])
            load_wt(IB - 1)
            nc.sync.dma_start(xc1[:, 1:], xt_r[:, 1:, NCHUNK:2 * NCHUNK])
            nc.sync.dma_start(bias_s[:], b_d[:])

            F = {}

            def features(chunk, xch):
                """Emit feature ops for all i-blocks of one chunk."""
                N = NCHUNK
                for ib in range(IB):
                    xcb = xch[:, ib]

                    def single(f):
                        t = f_pool.tile([P, NCHUNK], f16, tag=f"F_{ib}_{f}",
                                        name=f"F_{ib}_{f}")
                        F[ib, f] = t
                        return t

                    # A = [x-1/2 | x+1/2], A2 = squares, A3 = cubes (fp16)
                    A = tmp_pool.tile([P, 2 * NCHUNK], f16, tag="A", name="A")
                    nc.scalar.activation(A[:, :N], xcb[:], Act.Copy, bias=-0.5)
                    nc.scalar.activation(A[:, N:], xcb[:], Act.Copy, bias=0.5)
                    A2 = tmp_pool.tile([P, 2 * NCHUNK], f16, tag="A2", name="A2")
                    nc.scalar.activation(A2[:, :N], xcb[:], Act.Square, bias=bm05[:])
                    nc.scalar.activation(A2[:, N:], xcb[:], Act.Square, bias=b05[:])
                    A3 = tmp_pool.tile([P, 2 * NCHUNK], f16, tag="A3", name="A3")
                    nc.vector.tensor_tensor(A3[:], A2[:], A[:], Op.mult)
                    # mask m = (x < 1), exact 0/1 in fp16
                    m = single(0)
                    nc.gpsimd.tensor_scalar(m[:], xcb[:], 1.0, None, Op.is_lt)
                    # f5 = m*relu((x-1/2)^3); f6 = min(x+1/2,0)^3 (self-masked)
                    nc.vector.scalar_tensor_tensor(single(5)[:], A3[:, :N], 0.0,
                                                   m[:], Op.max, Op.mult)
                    nc.gpsimd.tensor_scalar(single(6)[:], A3[:, N:], 0.0, None,
                                            Op.min)
                    # monomial chain: f1 = m*x, f2 = f1^2, f3 = f1*f2,
                    # f4 = relu(f3)
                    f1 = single(1)
                    nc.vector.tensor_tensor(f1[:], m[:], xcb[:], Op.mult)
                    f2 = single(2)
                    nc.scalar.activation(f2[:], f1[:], Act.Square)
                    f3 = single(3)
                    nc.vector.tensor_tensor(f3[:], f1[:], f2[:], Op.mult)
                    nc.gpsimd.tensor_scalar(single(4)[:], f3[:], 0.0, None, Op.max)
                    # f7 = silu(x)
                    nc.scalar.activation(single(7)[:], xcb[:], Act.Silu)

            def lhs(ib, f, ns, Fc):
                return Fc[ib, f][:, ns * P:(ns + 1) * P]

            def evict(chunk, ps, ns, pieces=1):
                c0 = chunk * NCHUNK
                r0 = c0 + ns * P
                W = O_SHARD // pieces
                o = out_pool.tile([P, O_SHARD], f32, tag="out", name="outt")
                for j in range(pieces):
                    nc.vector.tensor_tensor(o[:, j * W:(j + 1) * W],
                                            ps[:, j * W:(j + 1) * W],
                                            bias_s[:, j * W:(j + 1) * W], Op.add)
                    nc.sync.dma_start(y_d[r0:r0 + P, j * W:(j + 1) * W],
                                      o[:, j * W:(j + 1) * W])

            # chunk 0 features (ib0's depend only on the tiny first x DMA)
            features(0, xc0)
            Fprev = dict(F)

            pending = []   # evictions deferred past the next chunk's features
            for chunk in range(CHUNKS):
                Fc = Fprev
                # -- matmuls. Chunk 0 runs k-major over both 128-batch
                # subtiles so each weight slab feeds two matmuls the moment
                # its DMA lands (PE stays ahead of the initial weight
                # stream); later chunks run the subtiles serially so
                # group-0's eviction overlaps group-1's matmuls --
                if chunk == 0:
                    pss = [psum_out.tile([P, O_SHARD], f32, tag=f"psout{ns}",
                                         name=f"psout{ns}", bufs=2)
                           for ns in range(NSUB)]
                    for k, (ib, f) in enumerate(
                            (ib, f) for ib in range(IB) for f in range(NF)):
                        for ns in range(NSUB):
                            nc.tensor.matmul(
                                pss[ns][:], lhs(ib, f, ns, Fc), wt[ib][:, f],
                                start=(k == 0), stop=(k == KT - 1))
                    pending = [(chunk, pss[0], 0), (chunk, pss[1], 1)]
                else:
                    for ns in range(NSUB):
                        ps = psum_out.tile([P, O_SHARD], f32, tag=f"psout{ns}",
                                           name=f"psout{ns}", bufs=2)
                        for k, (ib, f) in enumerate(
                                (ib, f) for ib in range(IB) for f in range(NF)):
                            nc.tensor.matmul(
                                ps[:], lhs(ib, f, ns, Fc), wt[ib][:, f],
                                start=(k == 0), stop=(k == KT - 1))
                        if ns == 0:
                            # mid-chunk psum: evict immediately, overlaps ns1
                            evict(chunk, ps, 0)
                        else:
                            pending.append((chunk, ps, 1))

                # next chunk's x DMA + features BEFORE the deferred
                # evictions so the boundary dependency chain (A3/f1/f3 on
                # DVE) is not queued behind a psum wait
                if chunk + 1 < CHUNKS:
                    if chunk + 1 == 1:
                        xch = xc1
                    else:
                        c1 = (chunk + 1) * NCHUNK
                        xch = xc_pool.tile([P, IB, NCHUNK], f16, tag="xc",
                                           name="xc")
                        nc.sync.dma_start(xch[:], xt_r[:, :, c1:c1 + NCHUNK])
                    F = {}
                    features(chunk + 1, xch)
                    Fprev = dict(F)
                for (ec, eps, ens) in pending:
                    # last eviction of the run: pipeline DVE + DMA in pieces
                    last = (chunk == CHUNKS - 1)
                    evict(ec, eps, ens, pieces=4 if last else 1)
                pending = []

    nc.compile()
    return nc


def _fold_weights(coeff, w_base):
    """Fold the feature->basis matrix into coeff; returns (K, D_OUT) fp16."""
    T = _T6 / 6.0
    c64 = np.asarray(coeff).astype(np.float64)
    # Wf[f, i, o] = sum_b T[f, b] * coeff[o, i, b]
    Wf = np.einsum('fb,oib->fio', T, c64)
    W8 = np.concatenate([Wf, np.asarray(w_base).astype(np.float64).T[None]], axis=0)
    # pack K as (ib, f, p): row k = ib*(NF*P) + f*P + p  <->  W8[f, ib*P+p, o]
    Wt = W8.reshape(NF, IB, P, D_OUT).transpose(1, 0, 2, 3).reshape(KT * P, D_OUT)
    return Wt.astype(np.float16)


def _prep_x16(x):
    """Host-side clamp to [-1,1] in fp16 with exact mask semantics at +1:
    any x < 1 that fp16 would round to 1.0 is pinned one ulp below."""
    x = np.asarray(x, dtype=np.float32)
    x16 = np.clip(x, -1.0, 1.0).astype(np.float16)
    edge = np.float16(1.0 - 2.0 ** -11)
    fix = (x < 1.0) & (x16 >= 1.0)
    if fix.any():
        x16[fix] = edge
    return x16


def kernel(x, coeff, w_base, bias):
    global _PROGRAM
    from concourse.bass_utils import run_bass_kernel_spmd

    if _PROGRAM is None:
        _PROGRAM = _build_program()
    nc = _PROGRAM

    x16 = _prep_x16(x)
    Wt = _fold_weights(coeff, w_base)
    bias = np.asarray(bias, dtype=np.float32)

    in_maps = []
    for core in range(8):
        cn, co = divmod(core, MESH_O)
        in_maps.append({
            "xt": np.ascontiguousarray(x16[cn * N_SHARD:(cn + 1) * N_SHARD].T),
            "wt": np.ascontiguousarray(Wt[:, co * O_SHARD:(co + 1) * O_SHARD]),
            "biasb": np.ascontiguousarray(np.broadcast_to(
                bias[co * O_SHARD:(co + 1) * O_SHARD], (P, O_SHARD)).astype(np.float32)),
        })

    res = run_bass_kernel_spmd(nc, in_maps, list(range(8)))

    y = np.empty((N_FULL, D_OUT), dtype=np.float32)
    for core in range(8):
        cn, co = divmod(core, MESH_O)
        y[cn * N_SHARD:(cn + 1) * N_SHARD, co * O_SHARD:(co + 1) * O_SHARD] = \
            res.results[core]["y"]
    return y


# revision 4
# speedup vs baseline: 1.5362x; 1.1347x over previous
"""BSplineKAN layer kernel for 8 Trainium2 NeuronCores.

Math
----
The reference computes, per element x = clip(x, -1, 1):
    y[n,o] = sum_{i,b} basis_b(x[n,i]) * coeff[o,i,b]  +  silu(x) @ w_base.T + bias
where basis is the 7-function clamped cubic B-spline basis on knots
{-1(x4), -0.5, 0, 0.5, 1(x4)}.  A quirk of the reference recurrence: at
x == 1.0 exactly (all clamped x >= 1 inputs) the basis row is all ZERO.

On [-1, 1) the 7 basis functions are C^2 piecewise cubics with breakpoints
at {-0.5, 0, +0.5}; the 7-dim space they span has the center-anchored
truncated-power basis
    feats = [m, m*x, m*x^2, m*x^3, m*x_+^3, m*(x-1/2)_+^3, min(x+1/2,0)^3]
with m = (x < 1) the edge mask (all seven vanish at x == 1, reproducing
the reference's edge behavior exactly).  basis_b = T[f,b] @ feats with T
integer/6, exact.  T is folded into coeff on the host.  silu(x)
is itself a smooth function on [-1,1], so instead of an extra feature it
is least-squares-fit in the SAME 7-dim spline space (max resid 5.3e-5)
and folded into the weights; the x==1 edge (features vanish, silu(1)
=0.731) folds exactly as 0.731*(1-m): the constant part goes to the
bias, the -0.731*m part into the f0 weight row.  One fused fp16 matmul
    y[n,o] = sum_{i,f} F_f(x[n,i]) * W[f,i,o] + bias'
with K = 7*1024 = 7168 (vs 11*1024 for the two-half-window local basis
with an explicit silu feature: 36% fewer FLOPs for a ~3x larger - still
~2e-3, 10x under the gate - fp16 cancellation error).
fp16 operands with fp32 PSUM accumulation.

x ships as fp16, pre-clamped on the host.  Values in (1-2^-12, 1) that
fp16 would round to exactly 1.0 are pinned to 1-2^-11 so the on-chip
mask m = (x < 1) matches the reference's fp32 comparison; this halves
the x DMA and removes the on-chip clamp from every dependency chain.

Distribution: 4-way batch x 2-way d_out mesh over 8 cores.  Per core:
x arrives host-transposed as (1024, 2048) fp16, W-shard (8192, 512)
fp16 stays resident in SBUF, output (2048, 512) fp32.  Features are
computed on DVE/ACT/Pool, and TensorE runs back-to-back 64-tile
K-accumulations into PSUM at the fp16 roofline (~213 ns per
512-column matmul).  Startup DMA order + PE warm-up are tuned so the
first real matmul issues ~4 us in at full clock; evictions are emitted
after the next chunk's feature ops so they never block the boundary.
"""

import numpy as np

# ---- problem constants (hardcoded per contract) ----
N_FULL, D_IN, D_OUT = 8192, 1024, 1024
MESH_N, MESH_O = 4, 2                 # 4-way batch x 2-way d_out
N_SHARD = N_FULL // MESH_N            # 2048
O_SHARD = D_OUT // MESH_O             # 512
P = 128
NF = 7                                # 7 spline features (silu folded in)
IB = D_IN // P                        # 8 i-blocks
KT = IB * NF                          # 64 K-tiles
NCHUNK = 256                          # batch cols per pipeline chunk
NSUB = NCHUNK // P                    # 2
CHUNKS = N_SHARD // NCHUNK            # 8
N_WARM = 72                           # PE warm-up matmuls (p-state + DMA cover)

# basis_b = sum_f feats_f * T6[f, b] / 6; feats order:
# [m, m*x, m*x^2, m*x^3, m*relu(x)^3, m*relu(x-1/2)^3, min(x+1/2,0)^3]
_T6 = np.array([
    [0,    0,    1,    4,    1,    0,   0],
    [0,    0,   -6,    0,    6,    0,   0],
    [0,    0,   12,  -24,   12,    0,   0],
    [0,  -12,   28,  -24,    8,    0,   0],
    [0,   12,  -36,   48,  -36,   12,   0],
    [0,    0,    8,  -32,   72,  -96,  48],
    [-48, 96,  -72,   32,   -8,    0,   0],
], dtype=np.float64)

# silu(x) ~= sum_f SILU_FIT[f] * feats_f(x) on [-1, 1)  (max resid 5.3e-5)
_SILU_FIT = np.array([-5.30336056e-05, 5.00000000e-01, 2.55431861e-01,
                      2.08452191e-02, -4.16904381e-02, -2.79657411e-02,
                      2.79657403e-02], dtype=np.float64)
_SILU_AT_1 = 0.7310585786300049

_PROGRAM = None  # compiled Bass program, built once


def _build_program():
    import concourse.mybir as mybir
    import concourse.tile as tile
    from concourse import bacc

    f32 = mybir.dt.float32
    f16 = mybir.dt.float16
    Op = mybir.AluOpType
    Act = mybir.ActivationFunctionType

    nc = bacc.Bacc("TRN2", target_bir_lowering=False, debug=False)
    xt_d = nc.dram_tensor("xt", [D_IN, N_SHARD], f16, kind="ExternalInput").ap()
    w_d = nc.dram_tensor("wt", [KT * P, O_SHARD], f16, kind="ExternalInput").ap()
    b_d = nc.dram_tensor("biasb", [P, O_SHARD], f32, kind="ExternalInput").ap()
    y_d = nc.dram_tensor("y", [N_SHARD, O_SHARD], f32, kind="ExternalOutput").ap()

    with tile.TileContext(nc) as tc:
        with (
            tc.tile_pool(name="const", bufs=1) as const_pool,
            tc.tile_pool(name="wt", bufs=1) as wt_pool,
            tc.tile_pool(name="feat", bufs=2) as f_pool,
            tc.tile_pool(name="xc", bufs=2) as xc_pool,
            tc.tile_pool(name="tmp", bufs=2) as tmp_pool,
            tc.tile_pool(name="out", bufs=3) as out_pool,
            tc.tile_pool(name="pso", bufs=4, space="PSUM") as psum_out,
        ):
            # PE warm-up first: wz memset is the only dependency, so the
            # warm-up matmuls start ~0.4us in and ramp the PE p-state while
            # the startup DMAs stream.
            wz = const_pool.tile([P, P], f16, name="wz")
            nc.gpsimd.memset(wz[:], 0.0)
            pw = psum_out.tile([P, 64], f32, tag="pwarm", name="pwarm")
            for i in range(N_WARM):
                nc.tensor.matmul(pw[:], wz[:], wz[:, :64],
                                 start=(i == 0), stop=(i == N_WARM - 1))

            # tiny dummy activations so both ACT table sets load concurrently
            # with the initial DMAs instead of on the first feature's
            # critical path
            warm = const_pool.tile([P, 1], f32, name="warm")
            nc.gpsimd.memset(warm[:], 0.0)
            nc.scalar.activation(warm[:], warm[:], Act.Copy, bias=0.0)
            nc.scalar.activation(warm[:], warm[:], Act.Square)
            b05 = const_pool.tile([P, 1], f32, name="b05")
            nc.gpsimd.memset(b05[:], 0.5)
            bm05 = const_pool.tile([P, 1], f32, name="bm05")
            nc.gpsimd.memset(bm05[:], -0.5)
            bias_s = const_pool.tile([P, O_SHARD], f32)

            # startup DMA order minimizes time-to-first-real-matmul on the
            # serial DMA device: a tiny x slice for i-block 0, then weight
            # slab 0, then the rest of chunk-0 x, then the remaining slabs.
            # Chunk-1 x is split the same way and threaded between the last
            # slabs so chunk-1 features start before the boundary.
            xt_r = xt_d.rearrange("(ib p) n -> p ib n", p=P)
            xc0 = xc_pool.tile([P, IB, NCHUNK], f16, tag="xc", name="xc0")
            nc.sync.dma_start(xc0[:, :1], xt_r[:, :1, 0:NCHUNK])

            wt = {}
            def load_wt(ib):
                t = wt_pool.tile([P, NF, O_SHARD], f16, tag=f"wt_{ib}", name=f"wt_{ib}")
                r0 = ib * NF * P
                nc.sync.dma_start(
                    t[:], w_d[r0:r0 + NF * P, :].rearrange("(f p) o -> p f o", p=P))
                wt[ib] = t
            load_wt(0)
            nc.sync.dma_start(xc0[:, 1:2], xt_r[:, 1:2, 0:NCHUNK])
            load_wt(1)
            nc.sync.dma_start(xc0[:, 2:], xt_r[:, 2:, 0:NCHUNK])
            for ib in range(2, IB - 1):
                load_wt(ib)
            xc1 = xc_pool.tile([P, IB, NCHUNK], f16, tag="xc", name="xc1")
            nc.sync.dma_start(xc1[:, :1], xt_r[:, :1, NCHUNK:2 * NCHUNK])
            load_wt(IB - 1)
            nc.sync.dma_start(xc1[:, 1:], xt_r[:, 1:, NCHUNK:2 * NCHUNK])
            nc.sync.dma_start(bias_s[:], b_d[:])

            F = {}

            def features(chunk, xch):
                """Emit feature ops for all i-blocks of one chunk."""
                N = NCHUNK
                for ib in range(IB):
                    xcb = xch[:, ib]

                    def single(f):
                        t = f_pool.tile([P, NCHUNK], f16, tag=f"F_{ib}_{f}",
                                        name=f"F_{ib}_{f}")
                        F[ib, f] = t
                        return t

                    # A = [x-1/2 | x+1/2], A2 = squares, A3 = cubes (fp16)
                    A = tmp_pool.tile([P, 2 * NCHUNK], f16, tag="A", name="A")
                    nc.scalar.activation(A[:, :N], xcb[:], Act.Copy, bias=-0.5)
                    nc.scalar.activation(A[:, N:], xcb[:], Act.Copy, bias=0.5)
                    A2 = tmp_pool.tile([P, 2 * NCHUNK], f16, tag="A2", name="A2")
                    nc.scalar.activation(A2[:, :N], xcb[:], Act.Square, bias=bm05[:])
                    nc.scalar.activation(A2[:, N:], xcb[:], Act.Square, bias=b05[:])
                    A3 = tmp_pool.tile([P, 2 * NCHUNK], f16, tag="A3", name="A3")
                    nc.vector.tensor_tensor(A3[:], A2[:], A[:], Op.mult)
                    # mask m = (x < 1), exact 0/1 in fp16
                    m = single(0)
                    nc.gpsimd.tensor_scalar(m[:], xcb[:], 1.0, None, Op.is_lt)
                    # f5 = m*relu((x-1/2)^3); f6 = min(x+1/2,0)^3 (self-masked)
                    nc.vector.scalar_tensor_tensor(single(5)[:], A3[:, :N], 0.0,
                                                   m[:], Op.max, Op.mult)
                    nc.gpsimd.tensor_scalar(single(6)[:], A3[:, N:], 0.0, None,
                                            Op.min)
                    # monomial chain: f1 = m*x, f2 = f1^2, f3 = f1*f2,
                    # f4 = relu(f3)
                    f1 = single(1)
                    nc.vector.tensor_tensor(f1[:], m[:], xcb[:], Op.mult)
                    f2 = single(2)
                    nc.scalar.activation(f2[:], f1[:], Act.Square)
                    f3 = single(3)
                    nc.vector.tensor_tensor(f3[:], f1[:], f2[:], Op.mult)
                    nc.gpsimd.tensor_scalar(single(4)[:], f3[:], 0.0, None, Op.max)

            def lhs(ib, f, ns, Fc):
                return Fc[ib, f][:, ns * P:(ns + 1) * P]

            def evict(chunk, ps, ns, pieces=1):
                c0 = chunk * NCHUNK
                r0 = c0 + ns * P
                W = O_SHARD // pieces
                o = out_pool.tile([P, O_SHARD], f32, tag="out", name="outt")
                for j in range(pieces):
                    nc.vector.tensor_tensor(o[:, j * W:(j + 1) * W],
                                            ps[:, j * W:(j + 1) * W],
                                            bias_s[:, j * W:(j + 1) * W], Op.add)
                    nc.sync.dma_start(y_d[r0:r0 + P, j * W:(j + 1) * W],
                                      o[:, j * W:(j + 1) * W])

            # chunk 0 features (ib0's depend only on the tiny first x DMA)
            features(0, xc0)
            Fprev = dict(F)

            pending = []   # evictions deferred past the next chunk's features
            for chunk in range(CHUNKS):
                Fc = Fprev
                # -- matmuls. Chunk 0 runs k-major over both 128-batch
                # subtiles so each weight slab feeds two matmuls the moment
                # its DMA lands (PE stays ahead of the initial weight
                # stream); later chunks run the subtiles serially so
                # group-0's eviction overlaps group-1's matmuls --
                if chunk == 0:
                    pss = [psum_out.tile([P, O_SHARD], f32, tag=f"psout{ns}",
                                         name=f"psout{ns}", bufs=2)
                           for ns in range(NSUB)]
                    for k, (ib, f) in enumerate(
                            (ib, f) for ib in range(IB) for f in range(NF)):
                        for ns in range(NSUB):
                            nc.tensor.matmul(
                                pss[ns][:], lhs(ib, f, ns, Fc), wt[ib][:, f],
                                start=(k == 0), stop=(k == KT - 1))
                    pending = [(chunk, pss[0], 0), (chunk, pss[1], 1)]
                else:
                    for ns in range(NSUB):
                        ps = psum_out.tile([P, O_SHARD], f32, tag=f"psout{ns}",
                                           name=f"psout{ns}", bufs=2)
                        for k, (ib, f) in enumerate(
                                (ib, f) for ib in range(IB) for f in range(NF)):
                            nc.tensor.matmul(
                                ps[:], lhs(ib, f, ns, Fc), wt[ib][:, f],
                                start=(k == 0), stop=(k == KT - 1))
                        if ns == 0:
                            # mid-chunk psum: evict immediately, overlaps ns1
                            evict(chunk, ps, 0)
                        else:
                            pending.append((chunk, ps, 1))

                # next chunk's x DMA + features BEFORE the deferred
                # evictions so the boundary dependency chain (A3/f1/f3 on
                # DVE) is not queued behind a psum wait
                if chunk + 1 < CHUNKS:
                    if chunk + 1 == 1:
                        xch = xc1
                    else:
                        c1 = (chunk + 1) * NCHUNK
                        xch = xc_pool.tile([P, IB, NCHUNK], f16, tag="xc",
                                           name="xc")
                        nc.sync.dma_start(xch[:], xt_r[:, :, c1:c1 + NCHUNK])
                    F = {}
                    features(chunk + 1, xch)
                    Fprev = dict(F)
                for (ec, eps, ens) in pending:
                    # last eviction of the run: pipeline DVE + DMA in pieces
                    last = (chunk == CHUNKS - 1)
                    evict(ec, eps, ens, pieces=2 if last else 1)
                pending = []

    nc.compile()
    return nc


def _fold_weights(coeff, w_base):
    """Fold the feature->basis matrix into coeff and absorb the silu/w_base
    path into the same 7 feature rows; returns ((K, D_OUT) fp16, bias_add)."""
    T = _T6 / 6.0
    c64 = np.asarray(coeff).astype(np.float64)
    wb = np.asarray(w_base).astype(np.float64)
    # Wf[f, i, o] = sum_b T[f, b] * coeff[o, i, b]  (+ silu fit via w_base)
    Wf = np.einsum('fb,oib->fio', T, c64)
    Wf += _SILU_FIT[:, None, None] * wb.T[None]
    Wf[0] -= _SILU_AT_1 * wb.T          # silu(1)*(1-m): -m part
    bias_add = _SILU_AT_1 * wb.sum(axis=1)   # constant part -> bias
    # pack K as (ib, f, p): row k = ib*(NF*P) + f*P + p  <->  Wf[f, ib*P+p, o]
    Wt = Wf.reshape(NF, IB, P, D_OUT).transpose(1, 0, 2, 3).reshape(KT * P, D_OUT)
    return Wt.astype(np.float16), bias_add


def _prep_x16(x):
    """Host-side clamp to [-1,1] in fp16 with exact mask semantics at +1:
    any x < 1 that fp16 would round to 1.0 is pinned one ulp below."""
    x = np.asarray(x, dtype=np.float32)
    x16 = np.clip(x, -1.0, 1.0).astype(np.float16)
    edge = np.float16(1.0 - 2.0 ** -11)
    fix = (x < 1.0) & (x16 >= 1.0)
    if fix.any():
        x16[fix] = edge
    return x16


def kernel(x, coeff, w_base, bias):
    global _PROGRAM
    from concourse.bass_utils import run_bass_kernel_spmd

    if _PROGRAM is None:
        _PROGRAM = _build_program()
    nc = _PROGRAM

    x16 = _prep_x16(x)
    Wt, bias_add = _fold_weights(coeff, w_base)
    bias = (np.asarray(bias, dtype=np.float64) + bias_add).astype(np.float32)

    in_maps = []
    for core in range(8):
        cn, co = divmod(core, MESH_O)
        in_maps.append({
            "xt": np.ascontiguousarray(x16[cn * N_SHARD:(cn + 1) * N_SHARD].T),
            "wt": np.ascontiguousarray(Wt[:, co * O_SHARD:(co + 1) * O_SHARD]),
            "biasb": np.ascontiguousarray(np.broadcast_to(
                bias[co * O_SHARD:(co + 1) * O_SHARD], (P, O_SHARD)).astype(np.float32)),
        })

    res = run_bass_kernel_spmd(nc, in_maps, list(range(8)))

    y = np.empty((N_FULL, D_OUT), dtype=np.float32)
    for core in range(8):
        cn, co = divmod(core, MESH_O)
        y[cn * N_SHARD:(cn + 1) * N_SHARD, co * O_SHARD:(co + 1) * O_SHARD] = \
            res.results[core]["y"]
    return y


# revision 5
# speedup vs baseline: 1.5379x; 1.0011x over previous
"""BSplineKAN layer kernel for 8 Trainium2 NeuronCores.

Math
----
The reference computes, per element x = clip(x, -1, 1):
    y[n,o] = sum_{i,b} basis_b(x[n,i]) * coeff[o,i,b]  +  silu(x) @ w_base.T + bias
where basis is the 7-function clamped cubic B-spline basis on knots
{-1(x4), -0.5, 0, 0.5, 1(x4)}.  A quirk of the reference recurrence: at
x == 1.0 exactly (all clamped x >= 1 inputs) the basis row is all ZERO.

On [-1, 1) the 7 basis functions are C^2 piecewise cubics with breakpoints
at {-0.5, 0, +0.5}; the 7-dim space they span has the center-anchored
truncated-power basis
    feats = [m, m*x, m*x^2, m*x^3, m*x_+^3, m*(x-1/2)_+^3, min(x+1/2,0)^3]
with m = (x < 1) the edge mask (all seven vanish at x == 1, reproducing
the reference's edge behavior exactly).  basis_b = T[f,b] @ feats with T
integer/6, exact.  T is folded into coeff on the host.  silu(x)
is itself a smooth function on [-1,1], so instead of an extra feature it
is least-squares-fit in the SAME 7-dim spline space (max resid 5.3e-5)
and folded into the weights; the x==1 edge (features vanish, silu(1)
=0.731) folds exactly as 0.731*(1-m): the constant part goes to the
bias, the -0.731*m part into the f0 weight row.  One fused fp16 matmul
    y[n,o] = sum_{i,f} F_f(x[n,i]) * W[f,i,o] + bias'
with K = 7*1024 = 7168 (vs 11*1024 for the two-half-window local basis
with an explicit silu feature: 36% fewer FLOPs for a ~3x larger - still
~2e-3, 10x under the gate - fp16 cancellation error).
fp16 operands with fp32 PSUM accumulation.

x ships as fp16, pre-clamped on the host.  Values in (1-2^-12, 1) that
fp16 would round to exactly 1.0 are pinned to 1-2^-11 so the on-chip
mask m = (x < 1) matches the reference's fp32 comparison; this halves
the x DMA and removes the on-chip clamp from every dependency chain.

Distribution: 4-way batch x 2-way d_out mesh over 8 cores.  Per core:
x arrives host-transposed as (1024, 2048) fp16, W-shard (8192, 512)
fp16 stays resident in SBUF, output (2048, 512) fp32.  Features are
computed on DVE/ACT/Pool, and TensorE runs back-to-back 64-tile
K-accumulations into PSUM at the fp16 roofline (~213 ns per
512-column matmul).  Startup DMA order + PE warm-up are tuned so the
first real matmul issues ~4 us in at full clock; evictions are emitted
after the next chunk's feature ops so they never block the boundary.
"""

import numpy as np

# ---- problem constants (hardcoded per contract) ----
N_FULL, D_IN, D_OUT = 8192, 1024, 1024
MESH_N, MESH_O = 4, 2                 # 4-way batch x 2-way d_out
N_SHARD = N_FULL // MESH_N            # 2048
O_SHARD = D_OUT // MESH_O             # 512
P = 128
NF = 7                                # 7 spline features (silu folded in)
IB = D_IN // P                        # 8 i-blocks
KT = IB * NF                          # 64 K-tiles
NCHUNK = 256                          # batch cols per pipeline chunk
NSUB = NCHUNK // P                    # 2
CHUNKS = N_SHARD // NCHUNK            # 8
N_WARM = 72                           # PE warm-up matmuls (p-state + DMA cover)

# basis_b = sum_f feats_f * T6[f, b] / 6; feats order:
# [m, m*x, m*x^2, m*x^3, m*relu(x)^3, m*relu(x-1/2)^3, min(x+1/2,0)^3]
_T6 = np.array([
    [0,    0,    1,    4,    1,    0,   0],
    [0,    0,   -6,    0,    6,    0,   0],
    [0,    0,   12,  -24,   12,    0,   0],
    [0,  -12,   28,  -24,    8,    0,   0],
    [0,   12,  -36,   48,  -36,   12,   0],
    [0,    0,    8,  -32,   72,  -96,  48],
    [-48, 96,  -72,   32,   -8,    0,   0],
], dtype=np.float64)

# silu(x) ~= sum_f SILU_FIT[f] * feats_f(x) on [-1, 1)  (max resid 5.3e-5)
_SILU_FIT = np.array([-5.30336056e-05, 5.00000000e-01, 2.55431861e-01,
                      2.08452191e-02, -4.16904381e-02, -2.79657411e-02,
                      2.79657403e-02], dtype=np.float64)
_SILU_AT_1 = 0.7310585786300049

_PROGRAM = None  # compiled Bass program, built once


def _build_program():
    import concourse.mybir as mybir
    import concourse.tile as tile
    from concourse import bacc

    f32 = mybir.dt.float32
    f16 = mybir.dt.float16
    Op = mybir.AluOpType
    Act = mybir.ActivationFunctionType

    nc = bacc.Bacc("TRN2", target_bir_lowering=False, debug=False)
    xt_d = nc.dram_tensor("xt", [D_IN, N_SHARD], f16, kind="ExternalInput").ap()
    w_d = nc.dram_tensor("wt", [KT * P, O_SHARD], f16, kind="ExternalInput").ap()
    b_d = nc.dram_tensor("biasb", [P, O_SHARD], f32, kind="ExternalInput").ap()
    y_d = nc.dram_tensor("y", [N_SHARD, O_SHARD], f32, kind="ExternalOutput").ap()

    with tile.TileContext(nc) as tc:
        with (
            tc.tile_pool(name="const", bufs=1) as const_pool,
            tc.tile_pool(name="wt", bufs=1) as wt_pool,
            tc.tile_pool(name="feat", bufs=2) as f_pool,
            tc.tile_pool(name="xc", bufs=2) as xc_pool,
            tc.tile_pool(name="tmp", bufs=2) as tmp_pool,
            tc.tile_pool(name="out", bufs=3) as out_pool,
            tc.tile_pool(name="pso", bufs=4, space="PSUM") as psum_out,
        ):
            # Startup DMAs are emitted FIRST so the sync engine issues them
            # immediately.  Each chunk-0 x i-block piece rides just before
            # its weight slab: the stream supplies a slab every ~2.7us while
            # the PE consumes one every ~3.0us, so after slab 0 lands the PE
            # never waits.  Chunk-1 x and the bias follow the last slab.
            xt_r = xt_d.rearrange("(ib p) n -> p ib n", p=P)
            xc0 = xc_pool.tile([P, IB, NCHUNK], f16, tag="xc", name="xc0")
            xc1 = xc_pool.tile([P, IB, NCHUNK], f16, tag="xc", name="xc1")

            wt = {}
            def load_wt(ib):
                t = wt_pool.tile([P, NF, O_SHARD], f16, tag=f"wt_{ib}", name=f"wt_{ib}")
                r0 = ib * NF * P
                nc.sync.dma_start(
                    t[:], w_d[r0:r0 + NF * P, :].rearrange("(f p) o -> p f o", p=P))
                wt[ib] = t
            for ib in range(IB):
                nc.sync.dma_start(xc0[:, ib:ib + 1], xt_r[:, ib:ib + 1, 0:NCHUNK])
                load_wt(ib)
            nc.sync.dma_start(xc1[:, :1], xt_r[:, :1, NCHUNK:2 * NCHUNK])
            nc.sync.dma_start(xc1[:, 1:], xt_r[:, 1:, NCHUNK:2 * NCHUNK])
            bias_s = const_pool.tile([P, O_SHARD], f32)
            nc.sync.dma_start(bias_s[:], b_d[:])

            # PE warm-up: burns the p-state ramp while the startup DMAs
            # stream, so real matmuls run at full clock from the start.
            wz = const_pool.tile([P, P], f16, name="wz")
            nc.gpsimd.memset(wz[:], 0.0)
            pw = psum_out.tile([P, 64], f32, tag="pwarm", name="pwarm")
            for i in range(N_WARM):
                nc.tensor.matmul(pw[:], wz[:], wz[:, :64],
                                 start=(i == 0), stop=(i == N_WARM - 1))

            F = {}

            def features(chunk, xch):
                """Emit feature ops for all i-blocks of one chunk."""
                N = NCHUNK
                for ib in range(IB):
                    xcb = xch[:, ib]

                    def single(f):
                        t = f_pool.tile([P, NCHUNK], f16, tag=f"F_{ib}_{f}",
                                        name=f"F_{ib}_{f}")
                        F[ib, f] = t
                        return t

                    # mask m = (x < 1), exact 0/1 in fp16 (Pool)
                    m = single(0)
                    nc.gpsimd.tensor_scalar(m[:], xcb[:], 1.0, None, Op.is_lt)
                    # A = [x-1/2 | x+1/2], A2 = A*A, A3 = A2*A (all DVE fp16;
                    # one in-order queue -> no cross-engine latency on the
                    # chunk-boundary critical chain, and ACT stays empty)
                    A = tmp_pool.tile([P, 2 * NCHUNK], f16, tag="A", name="A")
                    nc.vector.tensor_scalar(A[:, :N], xcb[:], -0.5, None, Op.add)
                    nc.vector.tensor_scalar(A[:, N:], xcb[:], 0.5, None, Op.add)
                    A2 = tmp_pool.tile([P, 2 * NCHUNK], f16, tag="A2", name="A2")
                    nc.vector.tensor_tensor(A2[:], A[:], A[:], Op.mult)
                    A3 = tmp_pool.tile([P, 2 * NCHUNK], f16, tag="A3", name="A3")
                    nc.vector.tensor_tensor(A3[:], A2[:], A[:], Op.mult)
                    # f5 = m*relu((x-1/2)^3); f6 = min(x+1/2,0)^3 (self-masked)
                    nc.vector.scalar_tensor_tensor(single(5)[:], A3[:, :N], 0.0,
                                                   m[:], Op.max, Op.mult)
                    nc.gpsimd.tensor_scalar(single(6)[:], A3[:, N:], 0.0, None,
                                            Op.min)
                    # monomial chain: f1 = m*x, f2 = f1^2, f3 = f1*f2,
                    # f4 = relu(f3)
                    f1 = single(1)
                    nc.vector.tensor_tensor(f1[:], m[:], xcb[:], Op.mult)
                    f2 = single(2)
                    nc.vector.tensor_tensor(f2[:], f1[:], f1[:], Op.mult)
                    f3 = single(3)
                    nc.vector.tensor_tensor(f3[:], f1[:], f2[:], Op.mult)
                    nc.gpsimd.tensor_scalar(single(4)[:], f3[:], 0.0, None, Op.max)

            def lhs(ib, f, ns, Fc):
                return Fc[ib, f][:, ns * P:(ns + 1) * P]

            def evict(chunk, ps, ns, pieces=1):
                c0 = chunk * NCHUNK
                r0 = c0 + ns * P
                W = O_SHARD // pieces
                o = out_pool.tile([P, O_SHARD], f32, tag="out", name="outt")
                for j in range(pieces):
                    nc.vector.tensor_tensor(o[:, j * W:(j + 1) * W],
                                            ps[:, j * W:(j + 1) * W],
                                            bias_s[:, j * W:(j + 1) * W], Op.add)
                    nc.sync.dma_start(y_d[r0:r0 + P, j * W:(j + 1) * W],
                                      o[:, j * W:(j + 1) * W])

            # chunk 0 features (ib0's depend only on the tiny first x DMA)
            features(0, xc0)
            Fprev = dict(F)

            pending = []   # evictions deferred past the next chunk's features
            for chunk in range(CHUNKS):
                Fc = Fprev
                # -- matmuls. Chunk 0 runs k-major over both 128-batch
                # subtiles so each weight slab feeds two matmuls the moment
                # its DMA lands (PE stays ahead of the initial weight
                # stream); later chunks run the subtiles serially so
                # group-0's eviction overlaps group-1's matmuls --
                if chunk == 0:
                    pss = [psum_out.tile([P, O_SHARD], f32, tag=f"psout{ns}",
                                         name=f"psout{ns}", bufs=2)
                           for ns in range(NSUB)]
                    for k, (ib, f) in enumerate(
                            (ib, f) for ib in range(IB) for f in range(NF)):
                        for ns in range(NSUB):
                            nc.tensor.matmul(
                                pss[ns][:], lhs(ib, f, ns, Fc), wt[ib][:, f],
                                start=(k == 0), stop=(k == KT - 1))
                    pending = [(chunk, pss[0], 0), (chunk, pss[1], 1)]
                else:
                    for ns in range(NSUB):
                        ps = psum_out.tile([P, O_SHARD], f32, tag=f"psout{ns}",
                                           name=f"psout{ns}", bufs=2)
                        for k, (ib, f) in enumerate(
                                (ib, f) for ib in range(IB) for f in range(NF)):
                            nc.tensor.matmul(
                                ps[:], lhs(ib, f, ns, Fc), wt[ib][:, f],
                                start=(k == 0), stop=(k == KT - 1))
                        if ns == 0:
                            # mid-chunk psum: evict immediately, overlaps ns1
                            evict(chunk, ps, 0)
                        else:
                            pending.append((chunk, ps, 1))

                # next chunk's x DMA + features BEFORE the deferred
                # evictions so the boundary dependency chain (A3/f1/f3 on
                # DVE) is not queued behind a psum wait
                if chunk + 1 < CHUNKS:
                    if chunk + 1 == 1:
                        xch = xc1
                    else:
                        c1 = (chunk + 1) * NCHUNK
                        xch = xc_pool.tile([P, IB, NCHUNK], f16, tag="xc",
                                           name="xc")
                        nc.sync.dma_start(xch[:], xt_r[:, :, c1:c1 + NCHUNK])
                    F = {}
                    features(chunk + 1, xch)
                    Fprev = dict(F)
                for (ec, eps, ens) in pending:
                    # last eviction of the run: pipeline DVE + DMA in pieces
                    last = (chunk == CHUNKS - 1)
                    evict(ec, eps, ens, pieces=2 if last else 1)
                pending = []

    nc.compile()
    return nc


def _fold_weights(coeff, w_base):
    """Fold the feature->basis matrix into coeff and absorb the silu/w_base
    path into the same 7 feature rows; returns ((K, D_OUT) fp16, bias_add)."""
    T = _T6 / 6.0
    c64 = np.asarray(coeff).astype(np.float64)
    wb = np.asarray(w_base).astype(np.float64)
    # Wf[f, i, o] = sum_b T[f, b] * coeff[o, i, b]  (+ silu fit via w_base)
    Wf = np.einsum('fb,oib->fio', T, c64)
    Wf += _SILU_FIT[:, None, None] * wb.T[None]
    Wf[0] -= _SILU_AT_1 * wb.T          # silu(1)*(1-m): -m part
    bias_add = _SILU_AT_1 * wb.sum(axis=1)   # constant part -> bias
    # pack K as (ib, f, p): row k = ib*(NF*P) + f*P + p  <->  Wf[f, ib*P+p, o]
    Wt = Wf.reshape(NF, IB, P, D_OUT).transpose(1, 0, 2, 3).reshape(KT * P, D_OUT)
    return Wt.astype(np.float16), bias_add


def _prep_x16(x):
    """Host-side clamp to [-1,1] in fp16 with exact mask semantics at +1:
    any x < 1 that fp16 would round to 1.0 is pinned one ulp below."""
    x = np.asarray(x, dtype=np.float32)
    x16 = np.clip(x, -1.0, 1.0).astype(np.float16)
    edge = np.float16(1.0 - 2.0 ** -11)
    fix = (x < 1.0) & (x16 >= 1.0)
    if fix.any():
        x16[fix] = edge
    return x16


def kernel(x, coeff, w_base, bias):
    global _PROGRAM
    from concourse.bass_utils import run_bass_kernel_spmd

    if _PROGRAM is None:
        _PROGRAM = _build_program()
    nc = _PROGRAM

    x16 = _prep_x16(x)
    Wt, bias_add = _fold_weights(coeff, w_base)
    bias = (np.asarray(bias, dtype=np.float64) + bias_add).astype(np.float32)

    in_maps = []
    for core in range(8):
        cn, co = divmod(core, MESH_O)
        in_maps.append({
            "xt": np.ascontiguousarray(x16[cn * N_SHARD:(cn + 1) * N_SHARD].T),
            "wt": np.ascontiguousarray(Wt[:, co * O_SHARD:(co + 1) * O_SHARD]),
            "biasb": np.ascontiguousarray(np.broadcast_to(
                bias[co * O_SHARD:(co + 1) * O_SHARD], (P, O_SHARD)).astype(np.float32)),
        })

    res = run_bass_kernel_spmd(nc, in_maps, list(range(8)))

    y = np.empty((N_FULL, D_OUT), dtype=np.float32)
    for core in range(8):
        cn, co = divmod(core, MESH_O)
        y[cn * N_SHARD:(cn + 1) * N_SHARD, co * O_SHARD:(co + 1) * O_SHARD] = \
            res.results[core]["y"]
    return y


# revision 7
# speedup vs baseline: 1.5462x; 1.0054x over previous
"""BSplineKAN layer kernel for 8 Trainium2 NeuronCores.

Math
----
The reference computes, per element x = clip(x, -1, 1):
    y[n,o] = sum_{i,b} basis_b(x[n,i]) * coeff[o,i,b]  +  silu(x) @ w_base.T + bias
where basis is the 7-function clamped cubic B-spline basis on knots
{-1(x4), -0.5, 0, 0.5, 1(x4)}.  A quirk of the reference recurrence: at
x == 1.0 exactly (all clamped x >= 1 inputs) the basis row is all ZERO.

On [-1, 1) the 7 basis functions are C^2 piecewise cubics with breakpoints
at {-0.5, 0, +0.5}; the 7-dim space they span has the center-anchored
truncated-power basis
    feats = [m, m*x, m*x^2, m*x^3, m*x_+^3, m*(x-1/2)_+^3, min(x+1/2,0)^3]
with m = (x < 1) the edge mask (all seven vanish at x == 1, reproducing
the reference's edge behavior exactly).  basis_b = T[f,b] @ feats with T
integer/6, exact.  T is folded into coeff on the host.  silu(x)
is itself a smooth function on [-1,1], so instead of an extra feature it
is least-squares-fit in the SAME 7-dim spline space (max resid 5.3e-5)
and folded into the weights; the x==1 edge (features vanish, silu(1)
=0.731) folds exactly as 0.731*(1-m): the constant part goes to the
bias, the -0.731*m part into the f0 weight row.  One fused fp16 matmul
    y[n,o] = sum_{i,f} F_f(x[n,i]) * W[f,i,o] + bias'
with K = 7*1024 = 7168 (vs 11*1024 for the two-half-window local basis
with an explicit silu feature: 36% fewer FLOPs for a ~3x larger - still
~2e-3, 10x under the gate - fp16 cancellation error).
fp16 operands with fp32 PSUM accumulation.

x ships as fp16, pre-clamped on the host.  Values in (1-2^-12, 1) that
fp16 would round to exactly 1.0 are pinned to 1-2^-11 so the on-chip
mask m = (x < 1) matches the reference's fp32 comparison; this halves
the x DMA and removes the on-chip clamp from every dependency chain.

Distribution: 4-way batch x 2-way d_out mesh over 8 cores.  Per core:
x arrives host-transposed as (1024, 2048) fp16, W-shard (8192, 512)
fp16 stays resident in SBUF, output (2048, 512) fp32.  Features are
computed on DVE/ACT/Pool, and TensorE runs back-to-back 64-tile
K-accumulations into PSUM at the fp16 roofline (~213 ns per
512-column matmul).  Startup DMA order + PE warm-up are tuned so the
first real matmul issues ~4 us in at full clock; evictions are emitted
after the next chunk's feature ops so they never block the boundary.
"""

import numpy as np

# ---- problem constants (hardcoded per contract) ----
N_FULL, D_IN, D_OUT = 8192, 1024, 1024
MESH_N, MESH_O = 4, 2                 # 4-way batch x 2-way d_out
N_SHARD = N_FULL // MESH_N            # 2048
O_SHARD = D_OUT // MESH_O             # 512
P = 128
NF = 7                                # 7 spline features (silu folded in)
IB = D_IN // P                        # 8 i-blocks
KT = IB * NF                          # 64 K-tiles
NCHUNK = 256                          # batch cols per pipeline chunk
NSUB = NCHUNK // P                    # 2
CHUNKS = N_SHARD // NCHUNK            # 8
N_WARM = 72                           # PE warm-up matmuls (p-state + DMA cover)

# basis_b = sum_f feats_f * T6[f, b] / 6; feats order:
# [m, m*x, m*x^2, m*x^3, m*relu(x)^3, m*relu(x-1/2)^3, min(x+1/2,0)^3]
_T6 = np.array([
    [0,    0,    1,    4,    1,    0,   0],
    [0,    0,   -6,    0,    6,    0,   0],
    [0,    0,   12,  -24,   12,    0,   0],
    [0,  -12,   28,  -24,    8,    0,   0],
    [0,   12,  -36,   48,  -36,   12,   0],
    [0,    0,    8,  -32,   72,  -96,  48],
    [-48, 96,  -72,   32,   -8,    0,   0],
], dtype=np.float64)

# silu(x) ~= sum_f SILU_FIT[f] * feats_f(x) on [-1, 1)  (max resid 5.3e-5)
_SILU_FIT = np.array([-5.30336056e-05, 5.00000000e-01, 2.55431861e-01,
                      2.08452191e-02, -4.16904381e-02, -2.79657411e-02,
                      2.79657403e-02], dtype=np.float64)
_SILU_AT_1 = 0.7310585786300049

_PROGRAM = None  # compiled Bass program, built once


def _build_program():
    import concourse.mybir as mybir
    import concourse.tile as tile
    from concourse import bacc

    f32 = mybir.dt.float32
    f16 = mybir.dt.float16
    Op = mybir.AluOpType
    Act = mybir.ActivationFunctionType

    nc = bacc.Bacc("TRN2", target_bir_lowering=False, debug=False)
    xt_d = nc.dram_tensor("xt", [D_IN, N_SHARD], f16, kind="ExternalInput").ap()
    w_d = nc.dram_tensor("wt", [KT * P, O_SHARD], f16, kind="ExternalInput").ap()
    b_d = nc.dram_tensor("biasb", [P, O_SHARD], f32, kind="ExternalInput").ap()
    y_d = nc.dram_tensor("y", [N_SHARD, O_SHARD], f32, kind="ExternalOutput").ap()

    with tile.TileContext(nc) as tc:
        with (
            tc.tile_pool(name="const", bufs=1) as const_pool,
            tc.tile_pool(name="wt", bufs=1) as wt_pool,
            tc.tile_pool(name="feat", bufs=2) as f_pool,
            tc.tile_pool(name="xc", bufs=2) as xc_pool,
            tc.tile_pool(name="tmp", bufs=2) as tmp_pool,
            tc.tile_pool(name="out", bufs=3) as out_pool,
            tc.tile_pool(name="pso", bufs=4, space="PSUM") as psum_out,
        ):
            # Startup DMAs are emitted FIRST so the sync engine issues them
            # immediately.  Each chunk-0 x i-block piece rides just before
            # its weight slab: the stream supplies a slab every ~2.7us while
            # the PE consumes one every ~3.0us, so after slab 0 lands the PE
            # never waits.  Chunk-1 x and the bias follow the last slab.
            xt_r = xt_d.rearrange("(ib p) n -> p ib n", p=P)
            xc0 = xc_pool.tile([P, IB, NCHUNK], f16, tag="xc", name="xc0")
            xc1 = xc_pool.tile([P, IB, NCHUNK], f16, tag="xc", name="xc1")

            wt = {}
            def load_wt(ib):
                t = wt_pool.tile([P, NF, O_SHARD], f16, tag=f"wt_{ib}", name=f"wt_{ib}")
                r0 = ib * NF * P
                nc.sync.dma_start(
                    t[:], w_d[r0:r0 + NF * P, :].rearrange("(f p) o -> p f o", p=P))
                wt[ib] = t
            for ib in range(IB - 1):
                nc.sync.dma_start(xc0[:, ib:ib + 1], xt_r[:, ib:ib + 1, 0:NCHUNK])
                load_wt(ib)
            nc.sync.dma_start(xc0[:, IB - 1:], xt_r[:, IB - 1:, 0:NCHUNK])
            # chunk-1 x rides before the last slab: slab 7 still arrives
            # before the PE needs it, and chunk-1's feature chain gets a
            # ~2.5us head start on the boundary
            nc.sync.dma_start(xc1[:, :1], xt_r[:, :1, NCHUNK:2 * NCHUNK])
            nc.sync.dma_start(xc1[:, 1:], xt_r[:, 1:, NCHUNK:2 * NCHUNK])
            load_wt(IB - 1)
            bias_s = const_pool.tile([P, O_SHARD], f32)
            nc.sync.dma_start(bias_s[:], b_d[:])

            # PE warm-up: burns the p-state ramp while the startup DMAs
            # stream, so real matmuls run at full clock from the start.
            wz = const_pool.tile([P, P], f16, name="wz")
            nc.gpsimd.memset(wz[:], 0.0)
            pw = psum_out.tile([P, 64], f32, tag="pwarm", name="pwarm")
            for i in range(N_WARM):
                nc.tensor.matmul(pw[:], wz[:], wz[:, :64],
                                 start=(i == 0), stop=(i == N_WARM - 1))

            F = {}

            def features(chunk, xch):
                """Emit feature ops for all i-blocks of one chunk."""
                N = NCHUNK
                for ib in range(IB):
                    xcb = xch[:, ib]

                    def single(f):
                        t = f_pool.tile([P, NCHUNK], f16, tag=f"F_{ib}_{f}",
                                        name=f"F_{ib}_{f}")
                        F[ib, f] = t
                        return t

                    # mask m = (x < 1), exact 0/1 in fp16 (Pool)
                    m = single(0)
                    nc.gpsimd.tensor_scalar(m[:], xcb[:], 1.0, None, Op.is_lt)
                    # A = [x-1/2 | x+1/2], A2 = A*A, A3 = A2*A (all DVE fp16;
                    # one in-order queue -> no cross-engine latency on the
                    # chunk-boundary critical chain, and ACT stays empty)
                    A = tmp_pool.tile([P, 2 * NCHUNK], f16, tag="A", name="A")
                    nc.vector.tensor_scalar(A[:, :N], xcb[:], -0.5, None, Op.add)
                    nc.vector.tensor_scalar(A[:, N:], xcb[:], 0.5, None, Op.add)
                    A2 = tmp_pool.tile([P, 2 * NCHUNK], f16, tag="A2", name="A2")
                    nc.vector.tensor_tensor(A2[:], A[:], A[:], Op.mult)
                    A3 = tmp_pool.tile([P, 2 * NCHUNK], f16, tag="A3", name="A3")
                    nc.vector.tensor_tensor(A3[:], A2[:], A[:], Op.mult)
                    # f5 = m*relu((x-1/2)^3); f6 = min(x+1/2,0)^3 (self-masked)
                    nc.vector.scalar_tensor_tensor(single(5)[:], A3[:, :N], 0.0,
                                                   m[:], Op.max, Op.mult)
                    nc.gpsimd.tensor_scalar(single(6)[:], A3[:, N:], 0.0, None,
                                            Op.min)
                    # monomial chain: f1 = m*x, f2 = f1^2, f3 = f1*f2,
                    # f4 = relu(f3)
                    f1 = single(1)
                    nc.vector.tensor_tensor(f1[:], m[:], xcb[:], Op.mult)
                    f2 = single(2)
                    nc.vector.tensor_tensor(f2[:], f1[:], f1[:], Op.mult)
                    f3 = single(3)
                    nc.vector.tensor_tensor(f3[:], f1[:], f2[:], Op.mult)
                    nc.gpsimd.tensor_scalar(single(4)[:], f3[:], 0.0, None, Op.max)

            def lhs(ib, f, ns, Fc):
                return Fc[ib, f][:, ns * P:(ns + 1) * P]

            def evict(chunk, ps, ns, pieces=1):
                c0 = chunk * NCHUNK
                r0 = c0 + ns * P
                W = O_SHARD // pieces
                o = out_pool.tile([P, O_SHARD], f32, tag="out", name="outt")
                for j in range(pieces):
                    nc.vector.tensor_tensor(o[:, j * W:(j + 1) * W],
                                            ps[:, j * W:(j + 1) * W],
                                            bias_s[:, j * W:(j + 1) * W], Op.add)
                    nc.sync.dma_start(y_d[r0:r0 + P, j * W:(j + 1) * W],
                                      o[:, j * W:(j + 1) * W])

            # chunk 0 features (ib0's depend only on the tiny first x DMA)
            features(0, xc0)
            Fprev = dict(F)

            pending = []   # evictions deferred past the next chunk's features
            for chunk in range(CHUNKS):
                Fc = Fprev
                # -- matmuls. Chunk 0 runs k-major over both 128-batch
                # subtiles so each weight slab feeds two matmuls the moment
                # its DMA lands (PE stays ahead of the initial weight
                # stream); later chunks run the subtiles serially so
                # group-0's eviction overlaps group-1's matmuls --
                if chunk == 0:
                    pss = [psum_out.tile([P, O_SHARD], f32, tag=f"psout{ns}",
                                         name=f"psout{ns}", bufs=2)
                           for ns in range(NSUB)]
                    for k, (ib, f) in enumerate(
                            (ib, f) for ib in range(IB) for f in range(NF)):
                        for ns in range(NSUB):
                            nc.tensor.matmul(
                                pss[ns][:], lhs(ib, f, ns, Fc), wt[ib][:, f],
                                start=(k == 0), stop=(k == KT - 1))
                    pending = [(chunk, pss[0], 0), (chunk, pss[1], 1)]
                else:
                    for ns in range(NSUB):
                        ps = psum_out.tile([P, O_SHARD], f32, tag=f"psout{ns}",
                                           name=f"psout{ns}", bufs=2)
                        for k, (ib, f) in enumerate(
                                (ib, f) for ib in range(IB) for f in range(NF)):
                            nc.tensor.matmul(
                                ps[:], lhs(ib, f, ns, Fc), wt[ib][:, f],
                                start=(k == 0), stop=(k == KT - 1))
                        if ns == 0:
                            # mid-chunk psum: evict immediately, overlaps ns1
                            evict(chunk, ps, 0)
                        else:
                            pending.append((chunk, ps, 1))

                # next chunk's x DMA + features BEFORE the deferred
                # evictions so the boundary dependency chain (A3/f1/f3 on
                # DVE) is not queued behind a psum wait
                if chunk + 1 < CHUNKS:
                    if chunk + 1 == 1:
                        xch = xc1
                    else:
                        c1 = (chunk + 1) * NCHUNK
                        xch = xc_pool.tile([P, IB, NCHUNK], f16, tag="xc",
                                           name="xc")
                        nc.sync.dma_start(xch[:], xt_r[:, :, c1:c1 + NCHUNK])
                    F = {}
                    features(chunk + 1, xch)
                    Fprev = dict(F)
                for (ec, eps, ens) in pending:
                    # last eviction of the run: pipeline DVE + DMA in pieces
                    last = (chunk == CHUNKS - 1)
                    evict(ec, eps, ens, pieces=2 if last else 1)
                pending = []

    nc.compile()
    return nc


def _fold_weights(coeff, w_base):
    """Fold the feature->basis matrix into coeff and absorb the silu/w_base
    path into the same 7 feature rows; returns ((K, D_OUT) fp16, bias_add)."""
    T = _T6 / 6.0
    c64 = np.asarray(coeff).astype(np.float64)
    wb = np.asarray(w_base).astype(np.float64)
    # Wf[f, i, o] = sum_b T[f, b] * coeff[o, i, b]  (+ silu fit via w_base)
    Wf = np.einsum('fb,oib->fio', T, c64)
    Wf += _SILU_FIT[:, None, None] * wb.T[None]
    Wf[0] -= _SILU_AT_1 * wb.T          # silu(1)*(1-m): -m part
    bias_add = _SILU_AT_1 * wb.sum(axis=1)   # constant part -> bias
    # pack K as (ib, f, p): row k = ib*(NF*P) + f*P + p  <->  Wf[f, ib*P+p, o]
    Wt = Wf.reshape(NF, IB, P, D_OUT).transpose(1, 0, 2, 3).reshape(KT * P, D_OUT)
    return Wt.astype(np.float16), bias_add


def _prep_x16(x):
    """Host-side clamp to [-1,1] in fp16 with exact mask semantics at +1:
    any x < 1 that fp16 would round to 1.0 is pinned one ulp below."""
    x = np.asarray(x, dtype=np.float32)
    x16 = np.clip(x, -1.0, 1.0).astype(np.float16)
    edge = np.float16(1.0 - 2.0 ** -11)
    fix = (x < 1.0) & (x16 >= 1.0)
    if fix.any():
        x16[fix] = edge
    return x16


def kernel(x, coeff, w_base, bias):
    global _PROGRAM
    from concourse.bass_utils import run_bass_kernel_spmd

    if _PROGRAM is None:
        _PROGRAM = _build_program()
    nc = _PROGRAM

    x16 = _prep_x16(x)
    Wt, bias_add = _fold_weights(coeff, w_base)
    bias = (np.asarray(bias, dtype=np.float64) + bias_add).astype(np.float32)

    in_maps = []
    for core in range(8):
        cn, co = divmod(core, MESH_O)
        in_maps.append({
            "xt": np.ascontiguousarray(x16[cn * N_SHARD:(cn + 1) * N_SHARD].T),
            "wt": np.ascontiguousarray(Wt[:, co * O_SHARD:(co + 1) * O_SHARD]),
            "biasb": np.ascontiguousarray(np.broadcast_to(
                bias[co * O_SHARD:(co + 1) * O_SHARD], (P, O_SHARD)).astype(np.float32)),
        })

    res = run_bass_kernel_spmd(nc, in_maps, list(range(8)))

    y = np.empty((N_FULL, D_OUT), dtype=np.float32)
    for core in range(8):
        cn, co = divmod(core, MESH_O)
        y[cn * N_SHARD:(cn + 1) * N_SHARD, co * O_SHARD:(co + 1) * O_SHARD] = \
            res.results[core]["y"]
    return y


# revision 8
# speedup vs baseline: 1.5525x; 1.0041x over previous
"""BSplineKAN layer kernel for 8 Trainium2 NeuronCores.

Math
----
The reference computes, per element x = clip(x, -1, 1):
    y[n,o] = sum_{i,b} basis_b(x[n,i]) * coeff[o,i,b]  +  silu(x) @ w_base.T + bias
where basis is the 7-function clamped cubic B-spline basis on knots
{-1(x4), -0.5, 0, 0.5, 1(x4)}.  A quirk of the reference recurrence: at
x == 1.0 exactly (all clamped x >= 1 inputs) the basis row is all ZERO.

On [-1, 1) the 7 basis functions are C^2 piecewise cubics with breakpoints
at {-0.5, 0, +0.5}; the 7-dim space they span has the center-anchored
truncated-power basis
    feats = [m, m*x, m*x^2, m*x^3, m*x_+^3, m*(x-1/2)_+^3, min(x+1/2,0)^3]
with m = (x < 1) the edge mask (all seven vanish at x == 1, reproducing
the reference's edge behavior exactly).  basis_b = T[f,b] @ feats with T
integer/6, exact.  T is folded into coeff on the host.  silu(x)
is itself a smooth function on [-1,1], so instead of an extra feature it
is least-squares-fit in the SAME 7-dim spline space (max resid 5.3e-5)
and folded into the weights; the x==1 edge (features vanish, silu(1)
=0.731) folds exactly as 0.731*(1-m): the constant part goes to the
bias, the -0.731*m part into the f0 weight row.  One fused fp16 matmul
    y[n,o] = sum_{i,f} F_f(x[n,i]) * W[f,i,o] + bias'
with K = 7*1024 = 7168 (vs 11*1024 for the two-half-window local basis
with an explicit silu feature: 36% fewer FLOPs for a ~3x larger - still
~2e-3, 10x under the gate - fp16 cancellation error).
fp16 operands with fp32 PSUM accumulation.

x ships as fp16, pre-clamped on the host.  Values in (1-2^-12, 1) that
fp16 would round to exactly 1.0 are pinned to 1-2^-11 so the on-chip
mask m = (x < 1) matches the reference's fp32 comparison; this halves
the x DMA and removes the on-chip clamp from every dependency chain.

Distribution: 4-way batch x 2-way d_out mesh over 8 cores.  Per core:
x arrives host-transposed as (1024, 2048) fp16, W-shard (8192, 512)
fp16 stays resident in SBUF, output (2048, 512) fp32.  Features are
computed on DVE/ACT/Pool, and TensorE runs back-to-back 64-tile
K-accumulations into PSUM at the fp16 roofline (~213 ns per
512-column matmul).  Startup DMA order + PE warm-up are tuned so the
first real matmul issues ~4 us in at full clock; evictions are emitted
after the next chunk's feature ops so they never block the boundary.
"""

import numpy as np

# ---- problem constants (hardcoded per contract) ----
N_FULL, D_IN, D_OUT = 8192, 1024, 1024
MESH_N, MESH_O = 4, 2                 # 4-way batch x 2-way d_out
N_SHARD = N_FULL // MESH_N            # 2048
O_SHARD = D_OUT // MESH_O             # 512
P = 128
NF = 7                                # 7 spline features (silu folded in)
IB = D_IN // P                        # 8 i-blocks
KT = IB * NF                          # 64 K-tiles
NCHUNK = 256                          # batch cols per pipeline chunk
NSUB = NCHUNK // P                    # 2
CHUNKS = N_SHARD // NCHUNK            # 8
N_WARM = 72                           # PE warm-up matmuls (p-state + DMA cover)

# basis_b = sum_f feats_f * T6[f, b] / 6; feats order:
# [m, m*x, m*x^2, m*x^3, m*relu(x)^3, m*relu(x-1/2)^3, min(x+1/2,0)^3]
_T6 = np.array([
    [0,    0,    1,    4,    1,    0,   0],
    [0,    0,   -6,    0,    6,    0,   0],
    [0,    0,   12,  -24,   12,    0,   0],
    [0,  -12,   28,  -24,    8,    0,   0],
    [0,   12,  -36,   48,  -36,   12,   0],
    [0,    0,    8,  -32,   72,  -96,  48],
    [-48, 96,  -72,   32,   -8,    0,   0],
], dtype=np.float64)

# silu(x) ~= sum_f SILU_FIT[f] * feats_f(x) on [-1, 1)  (max resid 5.3e-5)
_SILU_FIT = np.array([-5.30336056e-05, 5.00000000e-01, 2.55431861e-01,
                      2.08452191e-02, -4.16904381e-02, -2.79657411e-02,
                      2.79657403e-02], dtype=np.float64)
_SILU_AT_1 = 0.7310585786300049

_PROGRAM = None  # compiled Bass program, built once


def _build_program():
    import concourse.mybir as mybir
    import concourse.tile as tile
    from concourse import bacc

    f32 = mybir.dt.float32
    f16 = mybir.dt.float16
    Op = mybir.AluOpType
    Act = mybir.ActivationFunctionType

    nc = bacc.Bacc("TRN2", target_bir_lowering=False, debug=False)
    xt_d = nc.dram_tensor("xt", [D_IN, N_SHARD], f16, kind="ExternalInput").ap()
    w_d = nc.dram_tensor("wt", [KT * P, O_SHARD], f16, kind="ExternalInput").ap()
    b_d = nc.dram_tensor("biasb", [P, O_SHARD], f32, kind="ExternalInput").ap()
    y_d = nc.dram_tensor("y", [N_SHARD, O_SHARD], f32, kind="ExternalOutput").ap()

    with tile.TileContext(nc) as tc:
        with (
            tc.tile_pool(name="const", bufs=1) as const_pool,
            tc.tile_pool(name="wt", bufs=1) as wt_pool,
            tc.tile_pool(name="feat", bufs=2) as f_pool,
            tc.tile_pool(name="xc", bufs=2) as xc_pool,
            tc.tile_pool(name="tmp", bufs=2) as tmp_pool,
            tc.tile_pool(name="out", bufs=3) as out_pool,
            tc.tile_pool(name="pso", bufs=4, space="PSUM") as psum_out,
        ):
            # Startup DMAs are emitted FIRST so the sync engine issues them
            # immediately.  Each chunk-0 x i-block piece rides just before
            # its weight slab: the stream supplies a slab every ~2.7us while
            # the PE consumes one every ~3.0us, so after slab 0 lands the PE
            # never waits.  Chunk-1 x and the bias follow the last slab.
            xt_r = xt_d.rearrange("(ib p) n -> p ib n", p=P)
            xc0 = xc_pool.tile([P, IB, NCHUNK], f16, tag="xc", name="xc0")
            xc1 = xc_pool.tile([P, IB, NCHUNK], f16, tag="xc", name="xc1")

            wt = {}
            def load_wt(ib):
                t = wt_pool.tile([P, NF, O_SHARD], f16, tag=f"wt_{ib}", name=f"wt_{ib}")
                r0 = ib * NF * P
                nc.sync.dma_start(
                    t[:], w_d[r0:r0 + NF * P, :].rearrange("(f p) o -> p f o", p=P))
                wt[ib] = t
            for ib in range(IB - 1):
                nc.sync.dma_start(xc0[:, ib:ib + 1], xt_r[:, ib:ib + 1, 0:NCHUNK])
                load_wt(ib)
            nc.sync.dma_start(xc0[:, IB - 1:], xt_r[:, IB - 1:, 0:NCHUNK])
            # chunk-1 x rides before the last slab (chunk-1's feature chain
            # needs a ~2.5us head start on the boundary); slab 7 is split so
            # its first half still arrives before the PE reaches i-block 7
            nc.sync.dma_start(xc1[:, :1], xt_r[:, :1, NCHUNK:2 * NCHUNK])
            nc.sync.dma_start(xc1[:, 1:], xt_r[:, 1:, NCHUNK:2 * NCHUNK])
            ib7 = IB - 1
            wt7 = wt_pool.tile([P, NF, O_SHARD], f16, tag=f"wt_{ib7}",
                               name=f"wt_{ib7}")
            r7 = ib7 * NF * P
            nc.sync.dma_start(
                wt7[:, :4],
                w_d[r7:r7 + 4 * P, :].rearrange("(f p) o -> p f o", p=P))
            nc.sync.dma_start(
                wt7[:, 4:],
                w_d[r7 + 4 * P:r7 + NF * P, :].rearrange("(f p) o -> p f o", p=P))
            wt[ib7] = wt7
            bias_s = const_pool.tile([P, O_SHARD], f32)
            nc.sync.dma_start(bias_s[:], b_d[:])

            # PE warm-up: burns the p-state ramp while the startup DMAs
            # stream, so real matmuls run at full clock from the start.
            wz = const_pool.tile([P, P], f16, name="wz")
            nc.gpsimd.memset(wz[:], 0.0)
            pw = psum_out.tile([P, 64], f32, tag="pwarm", name="pwarm")
            for i in range(N_WARM):
                nc.tensor.matmul(pw[:], wz[:], wz[:, :64],
                                 start=(i == 0), stop=(i == N_WARM - 1))

            F = {}

            def features(chunk, xch):
                """Emit feature ops for all i-blocks of one chunk."""
                N = NCHUNK
                for ib in range(IB):
                    xcb = xch[:, ib]

                    def single(f):
                        t = f_pool.tile([P, NCHUNK], f16, tag=f"F_{ib}_{f}",
                                        name=f"F_{ib}_{f}")
                        F[ib, f] = t
                        return t

                    # mask m = (x < 1), exact 0/1 in fp16 (Pool)
                    m = single(0)
                    nc.gpsimd.tensor_scalar(m[:], xcb[:], 1.0, None, Op.is_lt)
                    # A = [x-1/2 | x+1/2], A2 = A*A, A3 = A2*A (all DVE fp16;
                    # one in-order queue -> no cross-engine latency on the
                    # chunk-boundary critical chain, and ACT stays empty)
                    A = tmp_pool.tile([P, 2 * NCHUNK], f16, tag="A", name="A")
                    nc.vector.tensor_scalar(A[:, :N], xcb[:], -0.5, None, Op.add)
                    nc.vector.tensor_scalar(A[:, N:], xcb[:], 0.5, None, Op.add)
                    A2 = tmp_pool.tile([P, 2 * NCHUNK], f16, tag="A2", name="A2")
                    nc.vector.tensor_tensor(A2[:], A[:], A[:], Op.mult)
                    A3 = tmp_pool.tile([P, 2 * NCHUNK], f16, tag="A3", name="A3")
                    nc.vector.tensor_tensor(A3[:], A2[:], A[:], Op.mult)
                    # f5 = m*relu((x-1/2)^3); f6 = min(x+1/2,0)^3 (self-masked)
                    nc.vector.scalar_tensor_tensor(single(5)[:], A3[:, :N], 0.0,
                                                   m[:], Op.max, Op.mult)
                    nc.gpsimd.tensor_scalar(single(6)[:], A3[:, N:], 0.0, None,
                                            Op.min)
                    # monomial chain: f1 = m*x, f2 = f1^2, f3 = f1*f2,
                    # f4 = relu(f3)
                    f1 = single(1)
                    nc.vector.tensor_tensor(f1[:], m[:], xcb[:], Op.mult)
                    f2 = single(2)
                    nc.vector.tensor_tensor(f2[:], f1[:], f1[:], Op.mult)
                    f3 = single(3)
                    nc.vector.tensor_tensor(f3[:], f1[:], f2[:], Op.mult)
                    nc.gpsimd.tensor_scalar(single(4)[:], f3[:], 0.0, None, Op.max)

            def lhs(ib, f, ns, Fc):
                return Fc[ib, f][:, ns * P:(ns + 1) * P]

            def evict(chunk, ps, ns, pieces=1):
                c0 = chunk * NCHUNK
                r0 = c0 + ns * P
                W = O_SHARD // pieces
                o = out_pool.tile([P, O_SHARD], f32, tag="out", name="outt")
                for j in range(pieces):
                    nc.vector.tensor_tensor(o[:, j * W:(j + 1) * W],
                                            ps[:, j * W:(j + 1) * W],
                                            bias_s[:, j * W:(j + 1) * W], Op.add)
                    nc.sync.dma_start(y_d[r0:r0 + P, j * W:(j + 1) * W],
                                      o[:, j * W:(j + 1) * W])

            # chunk 0 features (ib0's depend only on the tiny first x DMA)
            features(0, xc0)
            Fprev = dict(F)

            pending = []   # evictions deferred past the next chunk's features
            for chunk in range(CHUNKS):
                Fc = Fprev
                # -- matmuls. Chunk 0 runs k-major over both 128-batch
                # subtiles so each weight slab feeds two matmuls the moment
                # its DMA lands (PE stays ahead of the initial weight
                # stream); later chunks run the subtiles serially so
                # group-0's eviction overlaps group-1's matmuls --
                if chunk == 0:
                    pss = [psum_out.tile([P, O_SHARD], f32, tag=f"psout{ns}",
                                         name=f"psout{ns}", bufs=2)
                           for ns in range(NSUB)]
                    for k, (ib, f) in enumerate(
                            (ib, f) for ib in range(IB) for f in range(NF)):
                        for ns in range(NSUB):
                            nc.tensor.matmul(
                                pss[ns][:], lhs(ib, f, ns, Fc), wt[ib][:, f],
                                start=(k == 0), stop=(k == KT - 1))
                    pending = [(chunk, pss[0], 0), (chunk, pss[1], 1)]
                else:
                    for ns in range(NSUB):
                        ps = psum_out.tile([P, O_SHARD], f32, tag=f"psout{ns}",
                                           name=f"psout{ns}", bufs=2)
                        for k, (ib, f) in enumerate(
                                (ib, f) for ib in range(IB) for f in range(NF)):
                            nc.tensor.matmul(
                                ps[:], lhs(ib, f, ns, Fc), wt[ib][:, f],
                                start=(k == 0), stop=(k == KT - 1))
                        if ns == 0:
                            # mid-chunk psum: evict immediately, overlaps ns1
                            evict(chunk, ps, 0)
                        else:
                            pending.append((chunk, ps, 1))

                # next chunk's x DMA + features BEFORE the deferred
                # evictions so the boundary dependency chain (A3/f1/f3 on
                # DVE) is not queued behind a psum wait
                if chunk + 1 < CHUNKS:
                    if chunk + 1 == 1:
                        xch = xc1
                    else:
                        c1 = (chunk + 1) * NCHUNK
                        xch = xc_pool.tile([P, IB, NCHUNK], f16, tag="xc",
                                           name="xc")
                        nc.sync.dma_start(xch[:], xt_r[:, :, c1:c1 + NCHUNK])
                    F = {}
                    features(chunk + 1, xch)
                    Fprev = dict(F)
                for (ec, eps, ens) in pending:
                    # last eviction of the run: pipeline DVE + DMA in pieces
                    last = (chunk == CHUNKS - 1)
                    evict(ec, eps, ens, pieces=2 if last else 1)
                pending = []

    nc.compile()
    return nc


def _fold_weights(coeff, w_base):
    """Fold the feature->basis matrix into coeff and absorb the silu/w_base
    path into the same 7 feature rows; returns ((K, D_OUT) fp16, bias_add)."""
    T = _T6 / 6.0
    c64 = np.asarray(coeff).astype(np.float64)
    wb = np.asarray(w_base).astype(np.float64)
    # Wf[f, i, o] = sum_b T[f, b] * coeff[o, i, b]  (+ silu fit via w_base)
    Wf = np.einsum('fb,oib->fio', T, c64)
    Wf += _SILU_FIT[:, None, None] * wb.T[None]
    Wf[0] -= _SILU_AT_1 * wb.T          # silu(1)*(1-m): -m part
    bias_add = _SILU_AT_1 * wb.sum(axis=1)   # constant part -> bias
    # pack K as (ib, f, p): row k = ib*(NF*P) + f*P + p  <->  Wf[f, ib*P+p, o]
    Wt = Wf.reshape(NF, IB, P, D_OUT).transpose(1, 0, 2, 3).reshape(KT * P, D_OUT)
    return Wt.astype(np.float16), bias_add


def _prep_x16(x):
    """Host-side clamp to [-1,1] in fp16 with exact mask semantics at +1:
    any x < 1 that fp16 would round to 1.0 is pinned one ulp below."""
    x = np.asarray(x, dtype=np.float32)
    x16 = np.clip(x, -1.0, 1.0).astype(np.float16)
    edge = np.float16(1.0 - 2.0 ** -11)
    fix = (x < 1.0) & (x16 >= 1.0)
    if fix.any():
        x16[fix] = edge
    return x16


def kernel(x, coeff, w_base, bias):
    global _PROGRAM
    from concourse.bass_utils import run_bass_kernel_spmd

    if _PROGRAM is None:
        _PROGRAM = _build_program()
    nc = _PROGRAM

    x16 = _prep_x16(x)
    Wt, bias_add = _fold_weights(coeff, w_base)
    bias = (np.asarray(bias, dtype=np.float64) + bias_add).astype(np.float32)

    in_maps = []
    for core in range(8):
        cn, co = divmod(core, MESH_O)
        in_maps.append({
            "xt": np.ascontiguousarray(x16[cn * N_SHARD:(cn + 1) * N_SHARD].T),
            "wt": np.ascontiguousarray(Wt[:, co * O_SHARD:(co + 1) * O_SHARD]),
            "biasb": np.ascontiguousarray(np.broadcast_to(
                bias[co * O_SHARD:(co + 1) * O_SHARD], (P, O_SHARD)).astype(np.float32)),
        })

    res = run_bass_kernel_spmd(nc, in_maps, list(range(8)))

    y = np.empty((N_FULL, D_OUT), dtype=np.float32)
    for core in range(8):
        cn, co = divmod(core, MESH_O)
        y[cn * N_SHARD:(cn + 1) * N_SHARD, co * O_SHARD:(co + 1) * O_SHARD] = \
            res.results[core]["y"]
    return y
